# revision 2
# baseline (speedup 1.0000x reference)
"""Trainium2 Bass kernel for nn_Attention_35588099015465 — fp8 DoubleRow version.

Full GQA attention layer (QKV proj + per-head RMS norm + head-indexed rotary +
causal SDPA + out proj), sharded over 8 NeuronCores as DP(batch=2) x TP(kv=4).

Speed structure (vs the bf16 baseline):
  - QKV projections run as fp8e4m3 DoubleRow matmuls (K=256 packed per
    instruction, 0.5 cyc/col): pure-fp8 main products + hi/lo residual
    corrections restricted to the first 128 tokens.  Softmax averaging
    attenuates quantization noise ~1/sqrt(row length), so only short-context
    rows need the compensation (measured: full-pure rel-err 0.047 -> 0.0088
    with the 128-row prefix).
  - Biases enter via an extra DoubleRow matmul against a ones vector
    (hi/lo fp8 split keeps them exact to ~0.4%); evacuation rides ACT.
  - P = exp(S - 2) is written directly in fp8 (e4m3, bias -2 keeps the
    range in [0, 54]); Sigma and P@V run as DoubleRow with chunk-paired P.
    The first 128 tq columns use a bf16 P block + bf16 v chunk instead.
  - Scores and the output projection stay bf16 (q/k direction noise and
    y/Wproj quantization do not attenuate; measured over budget in fp8).
  - Phase B iterates tq-chunk-major (n outer, head inner) so the output
    projection + its DMA interleave with the ACT-bound softmax work.
"""

import numpy as np
import ml_dtypes

B, T, C = 2, 2048, 2048
N_HEAD, N_KV = 16, 4
D = 128
G = N_HEAD // N_KV  # 4
EPS = 1.1920928955078125e-07
KC = C // 128  # 16 contraction chunks
MT = T // 128  # 16 row chunks
NT = T // 512  # 4 col chunks
SX, SW, SV = 16.0, 1024.0, 32.0
S = SX * SW
BIAS_Q = S * S * D * EPS  # 4096.0 exactly
BIAS_K = S * S * EPS      # 32.0 exactly

F8 = ml_dtypes.float8_e4m3
BF16 = ml_dtypes.bfloat16

_CACHE = {}


def build_nc(dbg=False):
    import concourse.mybir as mybir
    import concourse.tile as tile
    from concourse import bacc

    dt = mybir.dt
    f32, bf16, f8 = dt.float32, dt.bfloat16, dt.float8e4
    AF = mybir.ActivationFunctionType
    DR = mybir.MatmulPerfMode.DoubleRow

    nc = bacc.Bacc("TRN2", target_bir_lowering=False, debug=False, num_devices=8)

    xt_d = nc.declare_dram_parameter("xt", [128, KC * 2048], f8, isOutput=False)
    xlo_d = nc.declare_dram_parameter("xlo", [128, KC * 128], f8, isOutput=False)
    wq_d = nc.declare_dram_parameter("wq", [128, KC * 1024], f8, isOutput=False)
    wk_d = nc.declare_dram_parameter("wk", [128, KC * 256], f8, isOutput=False)
    wv_d = nc.declare_dram_parameter("wv", [128, KC * 256], f8, isOutput=False)
    wp_d = nc.declare_dram_parameter("wp", [128, G * 2048], bf16, isOutput=False)
    bw_d = nc.declare_dram_parameter("bw", [128, 2 * 768], f8, isOutput=False)
    ident_d = nc.declare_dram_parameter("ident", [128, 128], bf16, isOutput=False)
    nm_d = nc.declare_dram_parameter("nm", [128, 128], bf16, isOutput=False)
    out_d = nc.declare_dram_parameter("out", [T, C], bf16, isOutput=True)
    if dbg:
        dqh_d = nc.declare_dram_parameter("dqh", [128, T], f32, isOutput=True)
        dkh_d = nc.declare_dram_parameter("dkh", [128, T], f32, isOutput=True)
        dv_d = nc.declare_dram_parameter("dv", [128, D], f32, isOutput=True)
        dp_d = nc.declare_dram_parameter("dp", [128, 512], f32, isOutput=True)
        dis_d = nc.declare_dram_parameter("dis", [1, T], f32, isOutput=True)
        dyt_d = nc.declare_dram_parameter("dyt", [128, T], f32, isOutput=True)

    with tile.TileContext(nc) as tc:
        with (
            tc.tile_pool(name="consts", bufs=1) as cpool,
            tc.tile_pool(name="persist", bufs=1) as ppool,
        ):
            ident = cpool.tile([128, 128], bf16, tag="ident")
            nc.sync.dma_start(ident[:], ident_d[:])
            nm = cpool.tile([128, 128], bf16, tag="nm")
            nc.sync.dma_start(nm[:], nm_d[:])
            bw = cpool.tile([128, 2, 768], f8, tag="bw")
            nc.sync.dma_start(bw[:], bw_d[:])
            ones_mov = cpool.tile([128, 2, 512], f8, tag="ones_mov")
            nc.vector.memset(ones_mov[:], 1.0)
            ones1 = cpool.tile([128, 1], bf16, tag="ones1")
            nc.vector.memset(ones1[:], 1.0)
            onesv = cpool.tile([128, 1], bf16, tag="onesv")
            nc.vector.memset(onesv[:], SV)
            ones8 = cpool.tile([128, 2, 16], f8, tag="ones8")
            nc.vector.memset(ones8[:], SV)
            nbias = cpool.tile([128, 1], f32, tag="nbias")
            nc.vector.memset(nbias[:], -2.0)
            biasq = cpool.tile([1, 1], f32, tag="biasq")
            nc.vector.memset(biasq[:], BIAS_Q)
            biask = cpool.tile([1, 1], f32, tag="biask")
            nc.vector.memset(biask[:], BIAS_K)

            # persistent across phases
            qh = [ppool.tile([128, T], bf16, tag="qh", bufs=G, name="qh") for _ in range(G)]
            kh = ppool.tile([128, T], bf16, tag="kh", name="kh")
            v8 = ppool.tile([128, KC, D], f8, tag="v8", name="v8")
            v0b = ppool.tile([128, D], bf16, tag="v0b", name="v0b")
            yT = [ppool.tile([128, T], bf16, tag="yT", bufs=G, name="yT") for _ in range(G)]
            wpt = ppool.tile([128, G, 2048], bf16, tag="wpt", name="wpt")

            # ---------------- Phase A: projections + norms ----------------
            with (
                tc.tile_pool(name="phA", bufs=1) as apool,
                tc.tile_pool(name="phA_ps", space="PSUM", bufs=4) as aps,
                tc.tile_pool(name="phA_ss", space="PSUM", bufs=2) as sps_pool,
                tc.tile_pool(name="phA_tp", space="PSUM", bufs=2) as tp_pool,
            ):
                xt = apool.tile([128, KC, 2048], f8, tag="xt", name="xt")
                xlo = apool.tile([128, KC, 128], f8, tag="xlo", name="xlo")
                wqt = apool.tile([128, KC, 1024], f8, tag="wqt", name="wqt")
                wkt = apool.tile([128, KC, 256], f8, tag="wkt", name="wkt")
                wvt = apool.tile([128, KC, 256], f8, tag="wvt", name="wvt")
                for k2 in range(KC // 2):
                    nc.sync.dma_start(xt[:, 2 * k2:2 * k2 + 2, :],
                                      xt_d[:, 4096 * k2:4096 * (k2 + 1)])
                    if k2 % 2 == 0:
                        nc.sync.dma_start(wqt[:, 2 * k2:2 * k2 + 4, :],
                                          wq_d[:, 2048 * k2:2048 * k2 + 4096])
                nc.sync.dma_start(xlo[:], xlo_d[:])
                nc.sync.dma_start(wkt[:], wk_d[:])
                nc.sync.dma_start(wvt[:], wv_d[:])
                for g in range(G):
                    nc.sync.dma_start(wpt[:, g:g + 1, :],
                                      wp_d[:, 2048 * g:2048 * (g + 1)])

                # PE warm-up during the input-DMA ramp
                for w in range(36):
                    wps = aps.tile([128, 512], f32, tag="proj", bufs=4, name="wps")
                    nc.tensor.matmul(wps[:, :128], lhsT=ident[:], rhs=ident[:],
                                     start=True, stop=True)

                def project(unit, dst, evac_scale):
                    """unit 0..3 = q head g, 4 = k, 5 = v; dst [128, T] bf16."""
                    if unit < G:
                        wt, hi0, lo0, bcol = wqt, 512 + 128 * unit, 128 * unit, 128 * unit
                    elif unit == 4:
                        wt, hi0, lo0, bcol = wkt, 128, 0, 512
                    else:
                        wt, hi0, lo0, bcol = wvt, 128, 0, 640
                    for n in range(NT):
                        ps = aps.tile([128, 512], f32, tag="proj", bufs=4)
                        nc.tensor.matmul(
                            ps[:], lhsT=bw[:, :, bcol:bcol + 128],
                            rhs=ones_mov[:], perf_mode=DR,
                            start=True, stop=False, skip_group_check=True)
                        for i in range(KC // 2):
                            nc.tensor.matmul(
                                ps[:], lhsT=wt[:, 2 * i:2 * i + 2, hi0:hi0 + 128],
                                rhs=xt[:, 2 * i:2 * i + 2, 512 * n:512 * (n + 1)],
                                perf_mode=DR, start=False,
                                stop=(n > 0 and i == KC // 2 - 1),
                                skip_group_check=True)
                        if n == 0:
                            for i in range(KC // 2):
                                nc.tensor.matmul(
                                    ps[:, 0:128],
                                    lhsT=wt[:, 2 * i:2 * i + 2, lo0:lo0 + 128],
                                    rhs=xt[:, 2 * i:2 * i + 2, 0:128],
                                    perf_mode=DR, start=False, stop=False,
                                    skip_group_check=True)
                            for i in range(KC // 2):
                                nc.tensor.matmul(
                                    ps[:, 0:128],
                                    lhsT=wt[:, 2 * i:2 * i + 2, hi0:hi0 + 128],
                                    rhs=xlo[:, 2 * i:2 * i + 2, :],
                                    perf_mode=DR, start=False,
                                    stop=(i == KC // 2 - 1),
                                    skip_group_check=True)
                        nc.scalar.activation(
                            dst[:, 512 * n:512 * (n + 1)], ps[:], AF.Copy,
                            scale=evac_scale)

                def norm_apply(src, dst, bias_ap):
                    """dst = src * broadcast(1/sqrt(scale-corrected sumsq)),
                    chunked per 512 cols so the chain pipelines across engines."""
                    sq = apool.tile([128, T], bf16, tag="sq", bufs=2, name="sq")
                    nc.vector.tensor_mul(sq[:], src[:], src[:])
                    srow = apool.tile([1, T], f32, tag="srow", bufs=2, name="srow")
                    crow = apool.tile([1, T], f32, tag="crow", bufs=2, name="crow")
                    for n in range(NT):
                        ssp = sps_pool.tile([1, 512], f32, tag="ss", bufs=2)
                        nc.tensor.matmul(
                            ssp[:], lhsT=ones1[:], rhs=sq[:, 512 * n:512 * (n + 1)],
                            start=True, stop=True)
                        nc.scalar.activation(
                            srow[:, 512 * n:512 * (n + 1)], ssp[:], AF.Sqrt,
                            bias=bias_ap, scale=(1.0 if bias_ap is biasq else 1.0 / D))
                        nc.vector.reciprocal_approx_fast(
                            crow[:, 512 * n:512 * (n + 1)],
                            srow[:, 512 * n:512 * (n + 1)])
                        bcc = apool.tile([128, 512], f32, tag="bcc", bufs=3, name="bcc")
                        nc.gpsimd.partition_broadcast(
                            bcc[:], crow[:, 512 * n:512 * (n + 1)])
                        nc.vector.tensor_mul(
                            dst[:, 512 * n:512 * (n + 1)],
                            src[:, 512 * n:512 * (n + 1)], bcc[:])

                # software-pipelined: project unit u+1 before norming unit u
                unit_order = [0, 4, 1, 2, 3, 5]   # q0, K, q1, q2, q3, V
                dsts, pending = {}, None
                for unit in unit_order:
                    if unit == 5:
                        vT = apool.tile([128, T], bf16, tag="vT", name="vT")
                        dsts[5] = vT
                        project(5, vT, SV / S)
                    else:
                        srcu = apool.tile([128, T], bf16, tag="qsb", bufs=3, name="qsb")
                        dsts[unit] = srcu
                        project(unit, srcu, 1.0)
                    if unit == 5:
                        for m in range(MT):
                            tp = tp_pool.tile([128, 128], bf16, tag="vtp", bufs=2)
                            nc.tensor.transpose(
                                tp[:], vT[:, 128 * m:128 * (m + 1)], ident[:])
                            if m % 2 == 0:
                                nc.vector.tensor_copy(v8[:, m:m + 1, :], tp[:])
                            else:
                                nc.scalar.activation(
                                    v8[:, m:m + 1, :], tp[:], AF.Copy, scale=1.0)
                            if m == 0:
                                nc.vector.tensor_copy(v0b[:], tp[:])
                    if pending is not None:
                        if pending == 4:
                            norm_apply(dsts[4], kh, biask)
                        else:
                            norm_apply(dsts[pending], qh[pending], biasq)
                    pending = unit
                # preload the exp table during phase A's ACT idle
                dume = apool.tile([1, 1], f32, tag="dume", bufs=1, name="dume")
                nc.scalar.activation(dume[:], biasq[:], AF.Exp)
                if dbg:
                    dcp = apool.tile([128, T], f32, tag="dcp", bufs=1, name="dcp")
                    nc.vector.tensor_copy(dcp[:], qh[0][:])
                    nc.sync.dma_start(dqh_d[:], dcp[:])
                    dcp2 = apool.tile([128, T], f32, tag="dcp2", bufs=1, name="dcp2")
                    nc.vector.tensor_copy(dcp2[:], kh[:])
                    nc.sync.dma_start(dkh_d[:], dcp2[:])
                    dcp3 = apool.tile([128, D], f32, tag="dcp3", bufs=1, name="dcp3")
                    nc.vector.tensor_copy(dcp3[:], v8[:, 0:1, :])
                    nc.sync.dma_start(dv_d[:], dcp3[:])

            # ---------------- Phase B: attention + interleaved out-proj ----------------
            with (
                tc.tile_pool(name="phB", bufs=1) as bpool,
                tc.tile_pool(name="phB_s", space="PSUM", bufs=2) as spool,
                tc.tile_pool(name="phB_y", space="PSUM", bufs=2) as ypool,
                tc.tile_pool(name="phB_sg", space="PSUM", bufs=2) as sgpool,
            ):
                isf = [bpool.tile([1, T], f32, tag="isf", bufs=G, name="isf") for _ in range(G)]

                # P00s: bf16 prefix P block per head (cols 0:512, only 0:128 nonzero)
                p00 = {}

                def scores_exp(g, n, pt):
                    """scores+exp for head g, tq window n, into pt [128, 4n+4, 512]."""
                    kmax = 4 * n + 3
                    for i in range(2 * n + 2):
                        sp = spool.tile([128, 2, 512], f32, tag="sp", bufs=2, name="sp")
                        for j in (0, 1):
                            kk = 2 * i + j
                            lo = max(128 * kk, 512 * n)
                            diag = 4 * n <= kk <= kmax
                            nc.tensor.matmul(
                                sp[:, j:j + 1, lo - 512 * n:512],
                                lhsT=kh[:, 128 * kk:128 * (kk + 1)],
                                rhs=qh[g][:, lo:512 * (n + 1)],
                                start=True, stop=not diag,
                                skip_group_check=True)
                            if diag:
                                dcol = 128 * kk - 512 * n
                                nc.tensor.matmul(
                                    sp[:, j:j + 1, dcol:dcol + 128],
                                    lhsT=ident[:], rhs=nm[:],
                                    start=False, stop=True,
                                    skip_group_check=True)
                        if i < 2 * n:
                            nc.scalar.activation(
                                pt[:, 2 * i:2 * i + 2, :], sp[:], AF.Exp,
                                bias=nbias[:])
                        else:
                            for j in (0, 1):
                                kk = 2 * i + j
                                lo = 128 * kk - 512 * n
                                nc.scalar.activation(
                                    pt[:, kk:kk + 1, lo:512], sp[:, j:j + 1, lo:512],
                                    AF.Exp, bias=nbias[:])
                                if kk == 0 and n == 0:
                                    ph = p00[g]
                                    nc.scalar.activation(
                                        ph[:, 0:128], sp[:, 0:1, 0:128], AF.Exp,
                                        bias=nbias[:])
                    # zero subdiagonal gaps of the two diagonal pairs
                    nc.gpsimd.memset(pt[:, 4 * n + 1:4 * n + 2, 0:128], 0.0)
                    nc.gpsimd.memset(pt[:, 4 * n + 3:4 * n + 4, 256:384], 0.0)

                def sig_pv(g, n, pt):
                    sgp = sgpool.tile([1, 512], f32, tag="sg", bufs=2)
                    yp = ypool.tile([128, 512], f32, tag="y", bufs=2)
                    first = True
                    if n == 0:
                        ph = p00[g]
                        nc.tensor.matmul(sgp[:, 0:128], lhsT=onesv[:], rhs=ph[:],
                                         start=True, stop=False,
                                         skip_group_check=True)
                        nc.tensor.matmul(yp[:, 0:128], lhsT=v0b[:], rhs=ph[:],
                                         start=True, stop=False,
                                         skip_group_check=True)
                        first = False
                    for i in range(2 * n + 2):
                        lo = max(256 * i, 512 * n, 128)
                        llo = lo - 512 * n
                        last = (i == 2 * n + 1)
                        nc.tensor.matmul(
                            sgp[:, llo:512], lhsT=ones8[:, :, 0:1],
                            rhs=pt[:, 2 * i:2 * i + 2, llo:512],
                            perf_mode=DR, start=first, stop=last,
                            skip_group_check=True)
                        nc.tensor.matmul(
                            yp[:, llo:512], lhsT=v8[:, 2 * i:2 * i + 2, :],
                            rhs=pt[:, 2 * i:2 * i + 2, llo:512],
                            perf_mode=DR, start=first, stop=last,
                            skip_group_check=True)
                        first = False
                    nc.vector.reciprocal_approx_fast(
                        isf[g][:, 512 * n:512 * (n + 1)], sgp[:])
                    bcn = bpool.tile([128, 512], f32, tag="bcn", bufs=2, name="bcn")
                    nc.gpsimd.partition_broadcast(
                        bcn[:], isf[g][:, 512 * n:512 * (n + 1)])
                    nc.vector.tensor_mul(
                        yT[g][:, 512 * n:512 * (n + 1)], yp[:], bcn[:])
                    if dbg and g == 0 and n == 0:
                        dpp = bpool.tile([128, 512], f32, tag="dpp", bufs=1, name="dpp")
                        nc.vector.tensor_copy(dpp[:], pt[:, 0:1, :])
                        nc.sync.dma_start(dp_d[:], dpp[:])

                out_sb = [bpool.tile([128, C], bf16, tag="osb", bufs=3, name="osb")
                          for _ in range(MT)]

                def cproj_cn(m, cns, late):
                    for cn in cns:
                        op = ypool.tile([128, 512], f32, tag="y", bufs=2)
                        for g in range(G):
                            nc.tensor.matmul(
                                op[:], lhsT=yT[g][:, 128 * m:128 * (m + 1)],
                                rhs=wpt[:, g:g + 1, 512 * cn:512 * (cn + 1)],
                                start=(g == 0), stop=(g == G - 1))
                        if late and cn % 2 == 1:
                            nc.scalar.activation(
                                out_sb[m][:, 512 * cn:512 * (cn + 1)], op[:],
                                AF.Copy, scale=1.0)
                        else:
                            nc.vector.tensor_copy(
                                out_sb[m][:, 512 * cn:512 * (cn + 1)], op[:])
                    if cns[-1] == NT - 1:
                        if late:
                            nc.sync.dma_start(
                                out_d[128 * m:128 * (m + 1), 0:1024],
                                out_sb[m][:, 0:1024])
                            nc.sync.dma_start(
                                out_d[128 * m:128 * (m + 1), 1024:2048],
                                out_sb[m][:, 1024:2048])
                        else:
                            nc.sync.dma_start(
                                out_d[128 * m:128 * (m + 1), :], out_sb[m][:])

                for n in range(NT):
                    for g in range(G):
                        if n == 0:
                            p00[g] = bpool.tile([128, 128], bf16, tag="p00",
                                                bufs=2, name="p00")
                        pt = bpool.tile([128, 4 * n + 4, 512], f8,
                                        tag=f"pt{n}", bufs=2, name="pt")
                        scores_exp(g, n, pt)
                        if n > 0:
                            cproj_cn(4 * (n - 1) + g, [0, 1], late=False)
                        sig_pv(g, n, pt)
                        if n > 0:
                            cproj_cn(4 * (n - 1) + g, [2, 3], late=False)
                    if n == 3:
                        for m in range(12, 16):
                            cproj_cn(m, [0, 1], late=True)
                            cproj_cn(m, [2, 3], late=True)
                if dbg:
                    nc.sync.dma_start(dis_d[:], isf[0][:])
                    dyt = bpool.tile([128, T], f32, tag="dyt", bufs=1, name="dyt")
                    nc.vector.tensor_copy(dyt[:], yT[0][:])
                    nc.sync.dma_start(dyt_d[:], dyt[:])

    nc.finalize()
    return nc


def _f8(x):
    return np.clip(x, -240.0, 240.0).astype(F8)


def host_inputs(x, Wq, bq, Wkv, bkv, Wproj):
    af = (1.0 / 1024.0) ** np.linspace(0.0, 1.0, D // 4, dtype=np.float32)
    af = np.concatenate([af, np.zeros(D // 4, dtype=np.float32)])  # (64,)
    ident = np.eye(128, dtype=np.float32).astype(BF16)
    p = np.arange(128)
    nm = np.where(p[None, :] >= p[:, None], 0.0, -30.0).astype(BF16)

    # x layouts per batch: xt [128, KC*2048] hi, xlo [128, KC*128]
    xts, xlos = [], []
    for b in range(B):
        xs = np.ascontiguousarray(x[b].T) * SX          # (C, T) scaled
        xc = xs.reshape(KC, 128, T)                      # chunk, p, t
        hi = _f8(xc)
        xts.append(np.ascontiguousarray(
            hi.transpose(1, 0, 2).reshape(128, KC * T)))
        lo = _f8((xc[:, :, :128] - hi[:, :, :128].astype(np.float32)))
        xlos.append(np.ascontiguousarray(
            lo.transpose(1, 0, 2).reshape(128, KC * 128)))

    def wsplit(Wm):
        """Wm (C, width) scaled -> [128, KC*(2*width)] with [lo|hi] per chunk."""
        width = Wm.shape[1]
        wc = (Wm * SW).reshape(KC, 128, width)
        hi = _f8(wc)
        lo = _f8(wc - hi.astype(np.float32))
        both = np.concatenate([lo, hi], axis=2)          # (KC, 128, 2*width)
        return np.ascontiguousarray(
            both.transpose(1, 0, 2).reshape(128, KC * 2 * width))

    def bias_rows(bvec):
        t = bvec * (S / 128.0)
        hi = _f8(t)
        lo = _f8(t - hi.astype(np.float32))
        return hi, lo

    in_maps = []
    for core in range(8):
        b, j = core // 4, core % 4
        wq_parts, bq_parts = [], []
        for g in range(G):
            h = G * j + g
            th = (h - j) * af
            cth, sth = np.cos(th).astype(np.float32), np.sin(th).astype(np.float32)
            R = np.zeros((D, D), np.float32)
            i = np.arange(64)
            R[i, i] = cth
            R[i, 64 + i] = sth
            R[64 + i, i] = -sth
            R[64 + i, 64 + i] = cth
            wq_parts.append(Wq[:, h * D:(h + 1) * D] @ R.T)
            bq_parts.append(bq[h * D:(h + 1) * D] @ R.T)
        Wq_j = np.concatenate(wq_parts, axis=1)          # (C, 512)
        Wk_j = Wkv[:, j * D:(j + 1) * D]
        Wv_j = Wkv[:, N_KV * D + j * D:N_KV * D + (j + 1) * D]
        bq_j = np.concatenate(bq_parts)
        bk_j = bkv[j * D:(j + 1) * D]
        bv_j = bkv[N_KV * D + j * D:N_KV * D + (j + 1) * D]
        bhi, blo = bias_rows(np.concatenate([bq_j, bk_j, bv_j]))   # (768,)
        bw = np.ascontiguousarray(np.broadcast_to(
            np.stack([bhi, blo], axis=0)[None, :, :], (128, 2, 768)
        ).reshape(128, 2 * 768))
        wp = np.ascontiguousarray(
            Wproj[512 * j:512 * (j + 1), :].reshape(G, 128, C)
            .transpose(1, 0, 2).reshape(128, G * C)).astype(BF16)
        in_maps.append({
            "xt": xts[b], "xlo": xlos[b],
            "wq": wsplit(Wq_j), "wk": wsplit(Wk_j), "wv": wsplit(Wv_j),
            "wp": wp, "bw": bw,
            "ident": ident, "nm": nm,
        })
    return in_maps


def assemble(parts, bproj):
    out = np.empty((B, T, C), np.float32)
    for b in range(B):
        out[b] = parts[4 * b].astype(np.float32)
        for c in range(1, 4):
            out[b] += parts[4 * b + c].astype(np.float32)
        out[b] += bproj[None, :]
    return out


def kernel(x, mask, Wq, bq, Wkv, bkv, Wproj, bproj):
    from concourse.bass_utils import run_bass_kernel_spmd

    x = np.asarray(x, np.float32)
    in_maps = host_inputs(
        x, np.asarray(Wq, np.float32), np.asarray(bq, np.float32),
        np.asarray(Wkv, np.float32), np.asarray(bkv, np.float32),
        np.asarray(Wproj, np.float32))
    if "nc" not in _CACHE:
        _CACHE["nc"] = build_nc()
    res = run_bass_kernel_spmd(_CACHE["nc"], in_maps, list(range(8)))
    parts = [res.results[c]["out"] for c in range(8)]
    return assemble(parts, np.asarray(bproj, np.float32))


# revision 3
# speedup vs baseline: 1.0090x; 1.0090x over previous
"""Trainium2 Bass kernel for nn_Attention_35588099015465 — fp8 DoubleRow version.

Full GQA attention layer (QKV proj + per-head RMS norm + head-indexed rotary +
causal SDPA + out proj), sharded over 8 NeuronCores as DP(batch=2) x TP(kv=4).

Speed structure (vs the bf16 baseline):
  - QKV projections run as fp8e4m3 DoubleRow matmuls (K=256 packed per
    instruction, 0.5 cyc/col): pure-fp8 main products + hi/lo residual
    corrections restricted to the first 128 tokens.  Softmax averaging
    attenuates quantization noise ~1/sqrt(row length), so only short-context
    rows need the compensation (measured: full-pure rel-err 0.047 -> 0.0088
    with the 128-row prefix).
  - Biases enter via an extra DoubleRow matmul against a ones vector
    (hi/lo fp8 split keeps them exact to ~0.4%); evacuation rides ACT.
  - P = exp(S - 2) is written directly in fp8 (e4m3, bias -2 keeps the
    range in [0, 54]); Sigma and P@V run as DoubleRow with chunk-paired P.
    The first 128 tq columns use a bf16 P block + bf16 v chunk instead.
  - Scores and the output projection stay bf16 (q/k direction noise and
    y/Wproj quantization do not attenuate; measured over budget in fp8).
  - Phase B iterates tq-chunk-major (n outer, head inner) so the output
    projection + its DMA interleave with the ACT-bound softmax work.
"""

import numpy as np
import ml_dtypes

B, T, C = 2, 2048, 2048
N_HEAD, N_KV = 16, 4
D = 128
G = N_HEAD // N_KV  # 4
EPS = 1.1920928955078125e-07
KC = C // 128  # 16 contraction chunks
MT = T // 128  # 16 row chunks
NT = T // 512  # 4 col chunks
SX, SW, SV = 16.0, 1024.0, 32.0
S = SX * SW
BIAS_Q = S * S * D * EPS  # 4096.0 exactly
BIAS_K = S * S * EPS      # 32.0 exactly

F8 = ml_dtypes.float8_e4m3
BF16 = ml_dtypes.bfloat16

_CACHE = {}


def build_nc(dbg=False):
    import concourse.mybir as mybir
    import concourse.tile as tile
    from concourse import bacc

    dt = mybir.dt
    f32, bf16, f8 = dt.float32, dt.bfloat16, dt.float8e4
    AF = mybir.ActivationFunctionType
    DR = mybir.MatmulPerfMode.DoubleRow

    nc = bacc.Bacc("TRN2", target_bir_lowering=False, debug=False, num_devices=8)

    xt_d = nc.declare_dram_parameter("xt", [128, KC * 2048], f8, isOutput=False)
    xlo_d = nc.declare_dram_parameter("xlo", [128, KC * 128], f8, isOutput=False)
    wq_d = nc.declare_dram_parameter("wq", [128, KC * 1024], f8, isOutput=False)
    wk_d = nc.declare_dram_parameter("wk", [128, KC * 256], f8, isOutput=False)
    wv_d = nc.declare_dram_parameter("wv", [128, KC * 256], f8, isOutput=False)
    wp_d = nc.declare_dram_parameter("wp", [128, G * 2048], bf16, isOutput=False)
    bw_d = nc.declare_dram_parameter("bw", [128, 2 * 768], f8, isOutput=False)
    ident_d = nc.declare_dram_parameter("ident", [128, 128], bf16, isOutput=False)
    nm_d = nc.declare_dram_parameter("nm", [128, 128], bf16, isOutput=False)
    out_d = nc.declare_dram_parameter("out", [T, C], bf16, isOutput=True)
    if dbg:
        dqh_d = nc.declare_dram_parameter("dqh", [128, T], f32, isOutput=True)
        dkh_d = nc.declare_dram_parameter("dkh", [128, T], f32, isOutput=True)
        dv_d = nc.declare_dram_parameter("dv", [128, D], f32, isOutput=True)
        dp_d = nc.declare_dram_parameter("dp", [128, 512], f32, isOutput=True)
        dis_d = nc.declare_dram_parameter("dis", [1, T], f32, isOutput=True)
        dyt_d = nc.declare_dram_parameter("dyt", [128, T], f32, isOutput=True)

    with tile.TileContext(nc) as tc:
        with (
            tc.tile_pool(name="consts", bufs=1) as cpool,
            tc.tile_pool(name="persist", bufs=1) as ppool,
        ):
            ident = cpool.tile([128, 128], bf16, tag="ident")
            nc.sync.dma_start(ident[:], ident_d[:])
            nm = cpool.tile([128, 128], bf16, tag="nm")
            nc.sync.dma_start(nm[:], nm_d[:])
            bw = cpool.tile([128, 2, 768], f8, tag="bw")
            nc.sync.dma_start(bw[:], bw_d[:])
            ones_mov = cpool.tile([128, 2, 512], f8, tag="ones_mov")
            nc.vector.memset(ones_mov[:], 1.0)
            ones1 = cpool.tile([128, 1], bf16, tag="ones1")
            nc.vector.memset(ones1[:], 1.0)
            onesv = cpool.tile([128, 1], bf16, tag="onesv")
            nc.vector.memset(onesv[:], SV)
            ones8 = cpool.tile([128, 2, 16], f8, tag="ones8")
            nc.vector.memset(ones8[:], SV)
            nbias = cpool.tile([128, 1], f32, tag="nbias")
            nc.vector.memset(nbias[:], -2.0)
            biasq = cpool.tile([1, 1], f32, tag="biasq")
            nc.vector.memset(biasq[:], BIAS_Q)
            biask = cpool.tile([1, 1], f32, tag="biask")
            nc.vector.memset(biask[:], BIAS_K)

            # persistent across phases
            qh = [ppool.tile([128, T], bf16, tag="qh", bufs=G, name="qh") for _ in range(G)]
            kh = ppool.tile([128, T], bf16, tag="kh", name="kh")
            v8 = ppool.tile([128, KC, D], f8, tag="v8", name="v8")
            v0b = ppool.tile([128, D], bf16, tag="v0b", name="v0b")
            yT = [ppool.tile([128, T], bf16, tag="yT", bufs=G, name="yT") for _ in range(G)]
            wpt = ppool.tile([128, G, 2048], bf16, tag="wpt", name="wpt")

            # ---------------- Phase A: projections + norms ----------------
            with (
                tc.tile_pool(name="phA", bufs=1) as apool,
                tc.tile_pool(name="phA_ps", space="PSUM", bufs=4) as aps,
                tc.tile_pool(name="phA_ss", space="PSUM", bufs=2) as sps_pool,
                tc.tile_pool(name="phA_tp", space="PSUM", bufs=2) as tp_pool,
            ):
                xt = apool.tile([128, KC, 2048], f8, tag="xt", name="xt")
                xlo = apool.tile([128, KC, 128], f8, tag="xlo", name="xlo")
                wqt = apool.tile([128, KC, 1024], f8, tag="wqt", name="wqt")
                wkt = apool.tile([128, KC, 256], f8, tag="wkt", name="wkt")
                wvt = apool.tile([128, KC, 256], f8, tag="wvt", name="wvt")
                for k2 in range(KC // 2):
                    nc.sync.dma_start(xt[:, 2 * k2:2 * k2 + 2, :],
                                      xt_d[:, 4096 * k2:4096 * (k2 + 1)])
                    if k2 % 2 == 0:
                        nc.sync.dma_start(wqt[:, 2 * k2:2 * k2 + 4, :],
                                          wq_d[:, 2048 * k2:2048 * k2 + 4096])
                nc.sync.dma_start(xlo[:], xlo_d[:])
                nc.sync.dma_start(wkt[:], wk_d[:])
                nc.sync.dma_start(wvt[:], wv_d[:])
                for g in range(G):
                    nc.sync.dma_start(wpt[:, g:g + 1, :],
                                      wp_d[:, 2048 * g:2048 * (g + 1)])

                # PE warm-up during the input-DMA ramp
                for w in range(36):
                    wps = aps.tile([128, 512], f32, tag="proj", bufs=4, name="wps")
                    nc.tensor.matmul(wps[:, :128], lhsT=ident[:], rhs=ident[:],
                                     start=True, stop=True)

                def project(unit, dst, evac_scale):
                    """unit 0..3 = q head g, 4 = k, 5 = v; dst [128, T] bf16."""
                    if unit < G:
                        wt, hi0, lo0, bcol = wqt, 512 + 128 * unit, 128 * unit, 128 * unit
                    elif unit == 4:
                        wt, hi0, lo0, bcol = wkt, 128, 0, 512
                    else:
                        wt, hi0, lo0, bcol = wvt, 128, 0, 640
                    for n in range(NT):
                        ps = aps.tile([128, 512], f32, tag="proj", bufs=4)
                        nc.tensor.matmul(
                            ps[:], lhsT=bw[:, :, bcol:bcol + 128],
                            rhs=ones_mov[:], perf_mode=DR,
                            start=True, stop=False, skip_group_check=True)
                        for i in range(KC // 2):
                            nc.tensor.matmul(
                                ps[:], lhsT=wt[:, 2 * i:2 * i + 2, hi0:hi0 + 128],
                                rhs=xt[:, 2 * i:2 * i + 2, 512 * n:512 * (n + 1)],
                                perf_mode=DR, start=False,
                                stop=(n > 0 and i == KC // 2 - 1),
                                skip_group_check=True)
                        if n == 0:
                            for i in range(KC // 2):
                                nc.tensor.matmul(
                                    ps[:, 0:128],
                                    lhsT=wt[:, 2 * i:2 * i + 2, lo0:lo0 + 128],
                                    rhs=xt[:, 2 * i:2 * i + 2, 0:128],
                                    perf_mode=DR, start=False, stop=False,
                                    skip_group_check=True)
                            for i in range(KC // 2):
                                nc.tensor.matmul(
                                    ps[:, 0:128],
                                    lhsT=wt[:, 2 * i:2 * i + 2, hi0:hi0 + 128],
                                    rhs=xlo[:, 2 * i:2 * i + 2, :],
                                    perf_mode=DR, start=False,
                                    stop=(i == KC // 2 - 1),
                                    skip_group_check=True)
                        nc.scalar.activation(
                            dst[:, 512 * n:512 * (n + 1)], ps[:], AF.Copy,
                            scale=evac_scale)

                def norm_apply(src, dst, bias_ap):
                    """dst = src * broadcast(1/sqrt(scale-corrected sumsq)),
                    chunked per 512 cols so the chain pipelines across engines."""
                    sq = apool.tile([128, T], bf16, tag="sq", bufs=2, name="sq")
                    nc.vector.tensor_mul(sq[:], src[:], src[:])
                    srow = apool.tile([1, T], f32, tag="srow", bufs=2, name="srow")
                    crow = apool.tile([1, T], f32, tag="crow", bufs=2, name="crow")
                    for n in range(NT):
                        ssp = sps_pool.tile([1, 512], f32, tag="ss", bufs=2)
                        nc.tensor.matmul(
                            ssp[:], lhsT=ones1[:], rhs=sq[:, 512 * n:512 * (n + 1)],
                            start=True, stop=True)
                        nc.scalar.activation(
                            srow[:, 512 * n:512 * (n + 1)], ssp[:], AF.Sqrt,
                            bias=bias_ap, scale=(1.0 if bias_ap is biasq else 1.0 / D))
                        nc.vector.reciprocal_approx_fast(
                            crow[:, 512 * n:512 * (n + 1)],
                            srow[:, 512 * n:512 * (n + 1)])
                        bcc = apool.tile([128, 512], f32, tag="bcc", bufs=3, name="bcc")
                        nc.gpsimd.partition_broadcast(
                            bcc[:], crow[:, 512 * n:512 * (n + 1)])
                        nc.vector.tensor_mul(
                            dst[:, 512 * n:512 * (n + 1)],
                            src[:, 512 * n:512 * (n + 1)], bcc[:])

                # software-pipelined: project unit u+1 before norming unit u
                unit_order = [0, 4, 1, 2, 3, 5]   # q0, K, q1, q2, q3, V
                dsts, pending = {}, None
                for unit in unit_order:
                    if unit == 5:
                        vT = apool.tile([128, T], bf16, tag="vT", name="vT")
                        dsts[5] = vT
                        project(5, vT, SV / S)
                    else:
                        srcu = apool.tile([128, T], bf16, tag="qsb", bufs=3, name="qsb")
                        dsts[unit] = srcu
                        project(unit, srcu, 1.0)
                    if unit == 5:
                        for m in range(MT):
                            tp = tp_pool.tile([128, 128], bf16, tag="vtp", bufs=2)
                            nc.tensor.transpose(
                                tp[:], vT[:, 128 * m:128 * (m + 1)], ident[:])
                            if m % 2 == 0:
                                nc.vector.tensor_copy(v8[:, m:m + 1, :], tp[:])
                            else:
                                nc.scalar.activation(
                                    v8[:, m:m + 1, :], tp[:], AF.Copy, scale=1.0)
                            if m == 0:
                                nc.vector.tensor_copy(v0b[:], tp[:])
                    if pending is not None:
                        if pending == 4:
                            norm_apply(dsts[4], kh, biask)
                        else:
                            norm_apply(dsts[pending], qh[pending], biasq)
                    pending = unit
                # preload the exp table during phase A's ACT idle
                dume = apool.tile([1, 1], f32, tag="dume", bufs=1, name="dume")
                nc.scalar.activation(dume[:], biasq[:], AF.Exp)
                if dbg:
                    dcp = apool.tile([128, T], f32, tag="dcp", bufs=1, name="dcp")
                    nc.vector.tensor_copy(dcp[:], qh[0][:])
                    nc.sync.dma_start(dqh_d[:], dcp[:])
                    dcp2 = apool.tile([128, T], f32, tag="dcp2", bufs=1, name="dcp2")
                    nc.vector.tensor_copy(dcp2[:], kh[:])
                    nc.sync.dma_start(dkh_d[:], dcp2[:])
                    dcp3 = apool.tile([128, D], f32, tag="dcp3", bufs=1, name="dcp3")
                    nc.vector.tensor_copy(dcp3[:], v8[:, 0:1, :])
                    nc.sync.dma_start(dv_d[:], dcp3[:])

            # ---------------- Phase B: attention + interleaved out-proj ----------------
            with (
                tc.tile_pool(name="phB", bufs=1) as bpool,
                tc.tile_pool(name="phB_s", space="PSUM", bufs=2) as spool,
                tc.tile_pool(name="phB_y", space="PSUM", bufs=2) as ypool,
                tc.tile_pool(name="phB_sg", space="PSUM", bufs=2) as sgpool,
            ):
                isf = [bpool.tile([1, T], f32, tag="isf", bufs=G, name="isf") for _ in range(G)]

                # P00s: bf16 prefix P block per head (cols 0:512, only 0:128 nonzero)
                p00 = {}

                def scores_exp(g, n, pt):
                    """scores+exp for head g, tq window n, into pt [128, 4n+4, 512]."""
                    kmax = 4 * n + 3
                    for i in range(2 * n + 2):
                        sp = spool.tile([128, 2, 512], f32, tag="sp", bufs=2, name="sp")
                        for j in (0, 1):
                            kk = 2 * i + j
                            lo = max(128 * kk, 512 * n)
                            diag = 4 * n <= kk <= kmax
                            nc.tensor.matmul(
                                sp[:, j:j + 1, lo - 512 * n:512],
                                lhsT=kh[:, 128 * kk:128 * (kk + 1)],
                                rhs=qh[g][:, lo:512 * (n + 1)],
                                start=True, stop=not diag,
                                skip_group_check=True)
                            if diag:
                                dcol = 128 * kk - 512 * n
                                nc.tensor.matmul(
                                    sp[:, j:j + 1, dcol:dcol + 128],
                                    lhsT=ident[:], rhs=nm[:],
                                    start=False, stop=True,
                                    skip_group_check=True)
                        if i < 2 * n:
                            nc.scalar.activation(
                                pt[:, 2 * i:2 * i + 2, :], sp[:], AF.Exp,
                                bias=nbias[:])
                        else:
                            for j in (0, 1):
                                kk = 2 * i + j
                                lo = 128 * kk - 512 * n
                                nc.scalar.activation(
                                    pt[:, kk:kk + 1, lo:512], sp[:, j:j + 1, lo:512],
                                    AF.Exp, bias=nbias[:])
                                if kk == 0 and n == 0:
                                    ph = p00[g]
                                    nc.scalar.activation(
                                        ph[:, 0:128], sp[:, 0:1, 0:128], AF.Exp,
                                        bias=nbias[:])
                    # zero subdiagonal gaps of the two diagonal pairs
                    nc.gpsimd.memset(pt[:, 4 * n + 1:4 * n + 2, 0:128], 0.0)
                    nc.gpsimd.memset(pt[:, 4 * n + 3:4 * n + 4, 256:384], 0.0)

                def sig_pv(g, n, pt):
                    sgp = sgpool.tile([1, 512], f32, tag="sg", bufs=2)
                    yp = ypool.tile([128, 512], f32, tag="y", bufs=2)
                    first = True
                    if n == 0:
                        ph = p00[g]
                        nc.tensor.matmul(sgp[:, 0:128], lhsT=onesv[:], rhs=ph[:],
                                         start=True, stop=False,
                                         skip_group_check=True)
                        nc.tensor.matmul(yp[:, 0:128], lhsT=v0b[:], rhs=ph[:],
                                         start=True, stop=False,
                                         skip_group_check=True)
                        first = False
                    for i in range(2 * n + 2):
                        lo = max(256 * i, 512 * n, 128)
                        llo = lo - 512 * n
                        last = (i == 2 * n + 1)
                        nc.tensor.matmul(
                            sgp[:, llo:512], lhsT=ones8[:, :, 0:1],
                            rhs=pt[:, 2 * i:2 * i + 2, llo:512],
                            perf_mode=DR, start=first, stop=last,
                            skip_group_check=True)
                        nc.tensor.matmul(
                            yp[:, llo:512], lhsT=v8[:, 2 * i:2 * i + 2, :],
                            rhs=pt[:, 2 * i:2 * i + 2, llo:512],
                            perf_mode=DR, start=first, stop=last,
                            skip_group_check=True)
                        first = False
                    nc.vector.reciprocal_approx_fast(
                        isf[g][:, 512 * n:512 * (n + 1)], sgp[:])
                    bcn = bpool.tile([128, 512], f32, tag="bcn", bufs=2, name="bcn")
                    nc.gpsimd.partition_broadcast(
                        bcn[:], isf[g][:, 512 * n:512 * (n + 1)])
                    nc.vector.tensor_mul(
                        yT[g][:, 512 * n:512 * (n + 1)], yp[:], bcn[:])
                    if dbg and g == 0 and n == 0:
                        dpp = bpool.tile([128, 512], f32, tag="dpp", bufs=1, name="dpp")
                        nc.vector.tensor_copy(dpp[:], pt[:, 0:1, :])
                        nc.sync.dma_start(dp_d[:], dpp[:])

                out_sb = [bpool.tile([128, C], bf16, tag="osb", bufs=3, name="osb")
                          for _ in range(MT)]

                def cproj_cn(m, cns, late):
                    for cn in cns:
                        op = ypool.tile([128, 512], f32, tag="y", bufs=2)
                        for g in range(G):
                            nc.tensor.matmul(
                                op[:], lhsT=yT[g][:, 128 * m:128 * (m + 1)],
                                rhs=wpt[:, g:g + 1, 512 * cn:512 * (cn + 1)],
                                start=(g == 0), stop=(g == G - 1))
                        if late and cn % 2 == 1:
                            nc.scalar.activation(
                                out_sb[m][:, 512 * cn:512 * (cn + 1)], op[:],
                                AF.Copy, scale=1.0)
                        else:
                            nc.vector.tensor_copy(
                                out_sb[m][:, 512 * cn:512 * (cn + 1)], op[:])
                    if late:
                        for cn in cns:
                            nc.sync.dma_start(
                                out_d[128 * m:128 * (m + 1), 512 * cn:512 * (cn + 1)],
                                out_sb[m][:, 512 * cn:512 * (cn + 1)])
                    elif cns[-1] == NT - 1:
                        nc.sync.dma_start(
                            out_d[128 * m:128 * (m + 1), :], out_sb[m][:])

                for n in range(NT):
                    if n == 0:
                        pts = []
                        for g in range(G):
                            p00[g] = bpool.tile([128, 128], bf16, tag="p00",
                                                bufs=4, name="p00")
                            pt = bpool.tile([128, 4, 512], f8,
                                            tag="pt0", bufs=4, name="pt")
                            pts.append(pt)
                            scores_exp(g, 0, pt)
                        for g in range(G):
                            sig_pv(g, 0, pts[g])
                        continue
                    for g in range(G):
                        pt = bpool.tile([128, 4 * n + 4, 512], f8,
                                        tag=f"pt{n}", bufs=2, name="pt")
                        scores_exp(g, n, pt)
                        cproj_cn(4 * (n - 1) + g, [0, 1], late=False)
                        sig_pv(g, n, pt)
                        cproj_cn(4 * (n - 1) + g, [2, 3], late=False)
                    if n == 3:
                        for m in range(12, 16):
                            cproj_cn(m, [0, 1], late=True)
                            cproj_cn(m, [2, 3], late=True)
                if dbg:
                    nc.sync.dma_start(dis_d[:], isf[0][:])
                    dyt = bpool.tile([128, T], f32, tag="dyt", bufs=1, name="dyt")
                    nc.vector.tensor_copy(dyt[:], yT[0][:])
                    nc.sync.dma_start(dyt_d[:], dyt[:])

    nc.finalize()
    return nc


def _f8(x):
    return np.clip(x, -240.0, 240.0).astype(F8)


def host_inputs(x, Wq, bq, Wkv, bkv, Wproj):
    af = (1.0 / 1024.0) ** np.linspace(0.0, 1.0, D // 4, dtype=np.float32)
    af = np.concatenate([af, np.zeros(D // 4, dtype=np.float32)])  # (64,)
    ident = np.eye(128, dtype=np.float32).astype(BF16)
    p = np.arange(128)
    nm = np.where(p[None, :] >= p[:, None], 0.0, -30.0).astype(BF16)

    # x layouts per batch: xt [128, KC*2048] hi, xlo [128, KC*128]
    xts, xlos = [], []
    for b in range(B):
        xs = np.ascontiguousarray(x[b].T) * SX          # (C, T) scaled
        xc = xs.reshape(KC, 128, T)                      # chunk, p, t
        hi = _f8(xc)
        xts.append(np.ascontiguousarray(
            hi.transpose(1, 0, 2).reshape(128, KC * T)))
        lo = _f8((xc[:, :, :128] - hi[:, :, :128].astype(np.float32)))
        xlos.append(np.ascontiguousarray(
            lo.transpose(1, 0, 2).reshape(128, KC * 128)))

    def wsplit(Wm):
        """Wm (C, width) scaled -> [128, KC*(2*width)] with [lo|hi] per chunk."""
        width = Wm.shape[1]
        wc = (Wm * SW).reshape(KC, 128, width)
        hi = _f8(wc)
        lo = _f8(wc - hi.astype(np.float32))
        both = np.concatenate([lo, hi], axis=2)          # (KC, 128, 2*width)
        return np.ascontiguousarray(
            both.transpose(1, 0, 2).reshape(128, KC * 2 * width))

    def bias_rows(bvec):
        t = bvec * (S / 128.0)
        hi = _f8(t)
        lo = _f8(t - hi.astype(np.float32))
        return hi, lo

    in_maps = []
    for core in range(8):
        b, j = core // 4, core % 4
        wq_parts, bq_parts = [], []
        for g in range(G):
            h = G * j + g
            th = (h - j) * af
            cth, sth = np.cos(th).astype(np.float32), np.sin(th).astype(np.float32)
            R = np.zeros((D, D), np.float32)
            i = np.arange(64)
            R[i, i] = cth
            R[i, 64 + i] = sth
            R[64 + i, i] = -sth
            R[64 + i, 64 + i] = cth
            wq_parts.append(Wq[:, h * D:(h + 1) * D] @ R.T)
            bq_parts.append(bq[h * D:(h + 1) * D] @ R.T)
        Wq_j = np.concatenate(wq_parts, axis=1)          # (C, 512)
        Wk_j = Wkv[:, j * D:(j + 1) * D]
        Wv_j = Wkv[:, N_KV * D + j * D:N_KV * D + (j + 1) * D]
        bq_j = np.concatenate(bq_parts)
        bk_j = bkv[j * D:(j + 1) * D]
        bv_j = bkv[N_KV * D + j * D:N_KV * D + (j + 1) * D]
        bhi, blo = bias_rows(np.concatenate([bq_j, bk_j, bv_j]))   # (768,)
        bw = np.ascontiguousarray(np.broadcast_to(
            np.stack([bhi, blo], axis=0)[None, :, :], (128, 2, 768)
        ).reshape(128, 2 * 768))
        wp = np.ascontiguousarray(
            Wproj[512 * j:512 * (j + 1), :].reshape(G, 128, C)
            .transpose(1, 0, 2).reshape(128, G * C)).astype(BF16)
        in_maps.append({
            "xt": xts[b], "xlo": xlos[b],
            "wq": wsplit(Wq_j), "wk": wsplit(Wk_j), "wv": wsplit(Wv_j),
            "wp": wp, "bw": bw,
            "ident": ident, "nm": nm,
        })
    return in_maps


def assemble(parts, bproj):
    out = np.empty((B, T, C), np.float32)
    for b in range(B):
        out[b] = parts[4 * b].astype(np.float32)
        for c in range(1, 4):
            out[b] += parts[4 * b + c].astype(np.float32)
        out[b] += bproj[None, :]
    return out


def kernel(x, mask, Wq, bq, Wkv, bkv, Wproj, bproj):
    from concourse.bass_utils import run_bass_kernel_spmd

    x = np.asarray(x, np.float32)
    in_maps = host_inputs(
        x, np.asarray(Wq, np.float32), np.asarray(bq, np.float32),
        np.asarray(Wkv, np.float32), np.asarray(bkv, np.float32),
        np.asarray(Wproj, np.float32))
    if "nc" not in _CACHE:
        _CACHE["nc"] = build_nc()
    res = run_bass_kernel_spmd(_CACHE["nc"], in_maps, list(range(8)))
    parts = [res.results[c]["out"] for c in range(8)]
    return assemble(parts, np.asarray(bproj, np.float32))


# revision 4
# speedup vs baseline: 1.0224x; 1.0133x over previous
"""Trainium2 Bass kernel for nn_Attention_35588099015465 — fp8 DoubleRow version.

Full GQA attention layer (QKV proj + per-head RMS norm + head-indexed rotary +
causal SDPA + out proj), sharded over 8 NeuronCores as DP(batch=2) x TP(kv=4).

Speed structure (vs the bf16 baseline):
  - QKV projections run as fp8e4m3 DoubleRow matmuls (K=256 packed per
    instruction, 0.5 cyc/col): pure-fp8 main products + hi/lo residual
    corrections restricted to the first 128 tokens.  Softmax averaging
    attenuates quantization noise ~1/sqrt(row length), so only short-context
    rows need the compensation (measured: full-pure rel-err 0.047 -> 0.0088
    with the 128-row prefix).
  - Biases enter via an extra DoubleRow matmul against a ones vector
    (hi/lo fp8 split keeps them exact to ~0.4%); evacuation rides ACT.
  - P = exp(S - 2) is written directly in fp8 (e4m3, bias -2 keeps the
    range in [0, 54]); Sigma and P@V run as DoubleRow with chunk-paired P.
    The first 128 tq columns use a bf16 P block + bf16 v chunk instead.
  - Scores and the output projection stay bf16 (q/k direction noise and
    y/Wproj quantization do not attenuate; measured over budget in fp8).
  - Phase B iterates tq-chunk-major (n outer, head inner) so the output
    projection + its DMA interleave with the ACT-bound softmax work.
"""

import numpy as np
import ml_dtypes

B, T, C = 2, 2048, 2048
N_HEAD, N_KV = 16, 4
D = 128
G = N_HEAD // N_KV  # 4
EPS = 1.1920928955078125e-07
KC = C // 128  # 16 contraction chunks
MT = T // 128  # 16 row chunks
NT = T // 512  # 4 col chunks
SX, SW, SV = 16.0, 1024.0, 32.0
S = SX * SW
BIAS_Q = S * S * D * EPS  # 4096.0 exactly
BIAS_K = S * S * EPS      # 32.0 exactly

F8 = ml_dtypes.float8_e4m3
BF16 = ml_dtypes.bfloat16

_CACHE = {}


def build_nc(dbg=False):
    import concourse.mybir as mybir
    import concourse.tile as tile
    from concourse import bacc

    dt = mybir.dt
    f32, bf16, f8 = dt.float32, dt.bfloat16, dt.float8e4
    AF = mybir.ActivationFunctionType
    DR = mybir.MatmulPerfMode.DoubleRow

    nc = bacc.Bacc("TRN2", target_bir_lowering=False, debug=False, num_devices=8)

    xt_d = nc.declare_dram_parameter("xt", [128, KC * 2048], f8, isOutput=False)
    xlo_d = nc.declare_dram_parameter("xlo", [128, KC * 128], f8, isOutput=False)
    wqh_d = nc.declare_dram_parameter("wqh", [128, KC * 512], f8, isOutput=False)
    wql_d = nc.declare_dram_parameter("wql", [128, KC * 512], f8, isOutput=False)
    wkh_d = nc.declare_dram_parameter("wkh", [128, KC * 128], f8, isOutput=False)
    wkl_d = nc.declare_dram_parameter("wkl", [128, KC * 128], f8, isOutput=False)
    wvh_d = nc.declare_dram_parameter("wvh", [128, KC * 128], f8, isOutput=False)
    wvl_d = nc.declare_dram_parameter("wvl", [128, KC * 128], f8, isOutput=False)
    wp_d = nc.declare_dram_parameter("wp", [128, G * 2048], bf16, isOutput=False)
    bw_d = nc.declare_dram_parameter("bw", [128, 2 * 768], f8, isOutput=False)
    ident_d = nc.declare_dram_parameter("ident", [128, 128], bf16, isOutput=False)
    nm_d = nc.declare_dram_parameter("nm", [128, 128], bf16, isOutput=False)
    out_d = nc.declare_dram_parameter("out", [T, C], bf16, isOutput=True)
    if dbg:
        dqh_d = nc.declare_dram_parameter("dqh", [128, T], f32, isOutput=True)
        dkh_d = nc.declare_dram_parameter("dkh", [128, T], f32, isOutput=True)
        dv_d = nc.declare_dram_parameter("dv", [128, D], f32, isOutput=True)
        dp_d = nc.declare_dram_parameter("dp", [128, 512], f32, isOutput=True)
        dis_d = nc.declare_dram_parameter("dis", [1, T], f32, isOutput=True)
        dyt_d = nc.declare_dram_parameter("dyt", [128, T], f32, isOutput=True)

    with tile.TileContext(nc) as tc:
        with (
            tc.tile_pool(name="consts", bufs=1) as cpool,
            tc.tile_pool(name="persist", bufs=1) as ppool,
        ):
            ident = cpool.tile([128, 128], bf16, tag="ident")
            nc.sync.dma_start(ident[:], ident_d[:])
            nm = cpool.tile([128, 128], bf16, tag="nm")
            nc.sync.dma_start(nm[:], nm_d[:])
            bw = cpool.tile([128, 2, 768], f8, tag="bw")
            nc.sync.dma_start(bw[:], bw_d[:])
            ones_mov = cpool.tile([128, 2, 512], f8, tag="ones_mov")
            nc.vector.memset(ones_mov[:], 1.0)
            ones1 = cpool.tile([128, 1], bf16, tag="ones1")
            nc.vector.memset(ones1[:], 1.0)
            onesv = cpool.tile([128, 1], bf16, tag="onesv")
            nc.vector.memset(onesv[:], SV)
            ones8 = cpool.tile([128, 2, 16], f8, tag="ones8")
            nc.vector.memset(ones8[:], SV)
            nbias = cpool.tile([128, 1], f32, tag="nbias")
            nc.vector.memset(nbias[:], -2.0)
            biasq = cpool.tile([1, 1], f32, tag="biasq")
            nc.vector.memset(biasq[:], BIAS_Q)
            biask = cpool.tile([1, 1], f32, tag="biask")
            nc.vector.memset(biask[:], BIAS_K)

            # persistent across phases
            qh = [ppool.tile([128, T], bf16, tag="qh", bufs=G, name="qh") for _ in range(G)]
            kh = ppool.tile([128, T], bf16, tag="kh", name="kh")
            v8 = ppool.tile([128, KC, D], f8, tag="v8", name="v8")
            v0b = ppool.tile([128, D], bf16, tag="v0b", name="v0b")
            yT = [ppool.tile([128, T], bf16, tag="yT", bufs=G, name="yT") for _ in range(G)]
            wpt = ppool.tile([128, G, 2048], bf16, tag="wpt", name="wpt")

            # ---------------- Phase A: projections + norms ----------------
            with (
                tc.tile_pool(name="phA", bufs=1) as apool,
                tc.tile_pool(name="phA_ps", space="PSUM", bufs=4) as aps,
                tc.tile_pool(name="phA_ss", space="PSUM", bufs=2) as sps_pool,
                tc.tile_pool(name="phA_tp", space="PSUM", bufs=2) as tp_pool,
            ):
                xt = apool.tile([128, KC, 2048], f8, tag="xt", name="xt")
                xlo = apool.tile([128, KC, 128], f8, tag="xlo", name="xlo")
                wqh = apool.tile([128, KC, 512], f8, tag="wqh", name="wqh")
                wql = apool.tile([128, KC, 512], f8, tag="wql", name="wql")
                wkh = apool.tile([128, KC, 128], f8, tag="wkh", name="wkh")
                wkl = apool.tile([128, KC, 128], f8, tag="wkl", name="wkl")
                wvh = apool.tile([128, KC, 128], f8, tag="wvh", name="wvh")
                wvl = apool.tile([128, KC, 128], f8, tag="wvl", name="wvl")
                for k2 in range(KC // 2):
                    nc.sync.dma_start(xt[:, 2 * k2:2 * k2 + 2, :],
                                      xt_d[:, 4096 * k2:4096 * (k2 + 1)])
                    if k2 % 2 == 0:
                        nc.sync.dma_start(wqh[:, 2 * k2:2 * k2 + 4, :],
                                          wqh_d[:, 1024 * k2:1024 * k2 + 2048])
                nc.sync.dma_start(xlo[:], xlo_d[:])
                nc.sync.dma_start(wkh[:], wkh_d[:])
                nc.sync.dma_start(wvh[:], wvh_d[:])
                nc.sync.dma_start(wql[:], wql_d[:])
                nc.sync.dma_start(wkl[:], wkl_d[:])
                nc.sync.dma_start(wvl[:], wvl_d[:])
                for g in range(G):
                    nc.sync.dma_start(wpt[:, g:g + 1, :],
                                      wp_d[:, 2048 * g:2048 * (g + 1)])

                # PE warm-up during the input-DMA ramp
                for w in range(36):
                    wps = aps.tile([128, 512], f32, tag="proj", bufs=4, name="wps")
                    nc.tensor.matmul(wps[:, :128], lhsT=ident[:], rhs=ident[:],
                                     start=True, stop=True)

                def project(unit, dst, evac_scale):
                    """unit 0..3 = q head g, 4 = k, 5 = v; dst [128, T] bf16."""
                    if unit < G:
                        wth, wtl, hi0, bcol = wqh, wql, 128 * unit, 128 * unit
                    elif unit == 4:
                        wth, wtl, hi0, bcol = wkh, wkl, 0, 512
                    else:
                        wth, wtl, hi0, bcol = wvh, wvl, 0, 640
                    for n in range(NT):
                        ps = aps.tile([128, 512], f32, tag="proj", bufs=4)
                        nc.tensor.matmul(
                            ps[:], lhsT=bw[:, :, bcol:bcol + 128],
                            rhs=ones_mov[:], perf_mode=DR,
                            start=True, stop=False, skip_group_check=True)
                        for i in range(KC // 2):
                            nc.tensor.matmul(
                                ps[:], lhsT=wth[:, 2 * i:2 * i + 2, hi0:hi0 + 128],
                                rhs=xt[:, 2 * i:2 * i + 2, 512 * n:512 * (n + 1)],
                                perf_mode=DR, start=False,
                                stop=(n > 0 and i == KC // 2 - 1),
                                skip_group_check=True)
                        if n == 0:
                            for i in range(KC // 2):
                                nc.tensor.matmul(
                                    ps[:, 0:128],
                                    lhsT=wtl[:, 2 * i:2 * i + 2, hi0:hi0 + 128],
                                    rhs=xt[:, 2 * i:2 * i + 2, 0:128],
                                    perf_mode=DR, start=False, stop=False,
                                    skip_group_check=True)
                            for i in range(KC // 2):
                                nc.tensor.matmul(
                                    ps[:, 0:128],
                                    lhsT=wth[:, 2 * i:2 * i + 2, hi0:hi0 + 128],
                                    rhs=xlo[:, 2 * i:2 * i + 2, :],
                                    perf_mode=DR, start=False,
                                    stop=(i == KC // 2 - 1),
                                    skip_group_check=True)
                        nc.scalar.activation(
                            dst[:, 512 * n:512 * (n + 1)], ps[:], AF.Copy,
                            scale=evac_scale)

                def norm_apply(src, dst, bias_ap):
                    """dst = src * broadcast(1/sqrt(scale-corrected sumsq)),
                    chunked per 512 cols so the chain pipelines across engines."""
                    sq = apool.tile([128, T], bf16, tag="sq", bufs=2, name="sq")
                    nc.vector.tensor_mul(sq[:], src[:], src[:])
                    srow = apool.tile([1, T], f32, tag="srow", bufs=2, name="srow")
                    crow = apool.tile([1, T], f32, tag="crow", bufs=2, name="crow")
                    for n in range(NT):
                        ssp = sps_pool.tile([1, 512], f32, tag="ss", bufs=2)
                        nc.tensor.matmul(
                            ssp[:], lhsT=ones1[:], rhs=sq[:, 512 * n:512 * (n + 1)],
                            start=True, stop=True)
                        nc.scalar.activation(
                            srow[:, 512 * n:512 * (n + 1)], ssp[:], AF.Sqrt,
                            bias=bias_ap, scale=(1.0 if bias_ap is biasq else 1.0 / D))
                        nc.vector.reciprocal_approx_fast(
                            crow[:, 512 * n:512 * (n + 1)],
                            srow[:, 512 * n:512 * (n + 1)])
                        bcc = apool.tile([128, 512], f32, tag="bcc", bufs=3, name="bcc")
                        nc.gpsimd.partition_broadcast(
                            bcc[:], crow[:, 512 * n:512 * (n + 1)])
                        nc.vector.tensor_mul(
                            dst[:, 512 * n:512 * (n + 1)],
                            src[:, 512 * n:512 * (n + 1)], bcc[:])

                # software-pipelined: project unit u+1 before norming unit u
                unit_order = [0, 4, 1, 2, 3, 5]   # q0, K, q1, q2, q3, V
                dsts, pending = {}, None
                for unit in unit_order:
                    if unit == 5:
                        vT = apool.tile([128, T], bf16, tag="vT", name="vT")
                        dsts[5] = vT
                        project(5, vT, SV / S)
                    else:
                        srcu = apool.tile([128, T], bf16, tag="qsb", bufs=3, name="qsb")
                        dsts[unit] = srcu
                        project(unit, srcu, 1.0)
                    if unit == 5:
                        for m in range(MT):
                            tp = tp_pool.tile([128, 128], bf16, tag="vtp", bufs=2)
                            nc.tensor.transpose(
                                tp[:], vT[:, 128 * m:128 * (m + 1)], ident[:])
                            if m % 2 == 0:
                                nc.vector.tensor_copy(v8[:, m:m + 1, :], tp[:])
                            else:
                                nc.scalar.activation(
                                    v8[:, m:m + 1, :], tp[:], AF.Copy, scale=1.0)
                            if m == 0:
                                nc.vector.tensor_copy(v0b[:], tp[:])
                    if pending is not None:
                        if pending == 4:
                            norm_apply(dsts[4], kh, biask)
                        else:
                            norm_apply(dsts[pending], qh[pending], biasq)
                    pending = unit
                # preload the exp table during phase A's ACT idle
                dume = apool.tile([1, 1], f32, tag="dume", bufs=1, name="dume")
                nc.scalar.activation(dume[:], biasq[:], AF.Exp)
                if dbg:
                    dcp = apool.tile([128, T], f32, tag="dcp", bufs=1, name="dcp")
                    nc.vector.tensor_copy(dcp[:], qh[0][:])
                    nc.sync.dma_start(dqh_d[:], dcp[:])
                    dcp2 = apool.tile([128, T], f32, tag="dcp2", bufs=1, name="dcp2")
                    nc.vector.tensor_copy(dcp2[:], kh[:])
                    nc.sync.dma_start(dkh_d[:], dcp2[:])
                    dcp3 = apool.tile([128, D], f32, tag="dcp3", bufs=1, name="dcp3")
                    nc.vector.tensor_copy(dcp3[:], v8[:, 0:1, :])
                    nc.sync.dma_start(dv_d[:], dcp3[:])

            # ---------------- Phase B: attention + interleaved out-proj ----------------
            with (
                tc.tile_pool(name="phB", bufs=1) as bpool,
                tc.tile_pool(name="phB_s", space="PSUM", bufs=2) as spool,
                tc.tile_pool(name="phB_y", space="PSUM", bufs=2) as ypool,
                tc.tile_pool(name="phB_sg", space="PSUM", bufs=2) as sgpool,
            ):
                isf = [bpool.tile([1, T], f32, tag="isf", bufs=G, name="isf") for _ in range(G)]

                # P00s: bf16 prefix P block per head (cols 0:512, only 0:128 nonzero)
                p00 = {}

                def scores_exp(g, n, pt):
                    """scores+exp for head g, tq window n, into pt [128, 4n+4, 512]."""
                    kmax = 4 * n + 3
                    for i in range(2 * n + 2):
                        sp = spool.tile([128, 2, 512], f32, tag="sp", bufs=2, name="sp")
                        for j in (0, 1):
                            kk = 2 * i + j
                            lo = max(128 * kk, 512 * n)
                            diag = 4 * n <= kk <= kmax
                            nc.tensor.matmul(
                                sp[:, j:j + 1, lo - 512 * n:512],
                                lhsT=kh[:, 128 * kk:128 * (kk + 1)],
                                rhs=qh[g][:, lo:512 * (n + 1)],
                                start=True, stop=not diag,
                                skip_group_check=True)
                            if diag:
                                dcol = 128 * kk - 512 * n
                                nc.tensor.matmul(
                                    sp[:, j:j + 1, dcol:dcol + 128],
                                    lhsT=ident[:], rhs=nm[:],
                                    start=False, stop=True,
                                    skip_group_check=True)
                        if i < 2 * n:
                            nc.scalar.activation(
                                pt[:, 2 * i:2 * i + 2, :], sp[:], AF.Exp,
                                bias=nbias[:])
                        else:
                            for j in (0, 1):
                                kk = 2 * i + j
                                lo = 128 * kk - 512 * n
                                nc.scalar.activation(
                                    pt[:, kk:kk + 1, lo:512], sp[:, j:j + 1, lo:512],
                                    AF.Exp, bias=nbias[:])
                                if kk == 0 and n == 0:
                                    ph = p00[g]
                                    nc.scalar.activation(
                                        ph[:, 0:128], sp[:, 0:1, 0:128], AF.Exp,
                                        bias=nbias[:])
                    # zero subdiagonal gaps of the two diagonal pairs
                    nc.gpsimd.memset(pt[:, 4 * n + 1:4 * n + 2, 0:128], 0.0)
                    nc.gpsimd.memset(pt[:, 4 * n + 3:4 * n + 4, 256:384], 0.0)

                def sig_pv(g, n, pt):
                    sgp = sgpool.tile([1, 512], f32, tag="sg", bufs=2)
                    yp = ypool.tile([128, 512], f32, tag="y", bufs=2)
                    first = True
                    if n == 0:
                        ph = p00[g]
                        nc.tensor.matmul(sgp[:, 0:128], lhsT=onesv[:], rhs=ph[:],
                                         start=True, stop=False,
                                         skip_group_check=True)
                        nc.tensor.matmul(yp[:, 0:128], lhsT=v0b[:], rhs=ph[:],
                                         start=True, stop=False,
                                         skip_group_check=True)
                        first = False
                    for i in range(2 * n + 2):
                        lo = max(256 * i, 512 * n, 128)
                        llo = lo - 512 * n
                        last = (i == 2 * n + 1)
                        nc.tensor.matmul(
                            sgp[:, llo:512], lhsT=ones8[:, :, 0:1],
                            rhs=pt[:, 2 * i:2 * i + 2, llo:512],
                            perf_mode=DR, start=first, stop=last,
                            skip_group_check=True)
                        nc.tensor.matmul(
                            yp[:, llo:512], lhsT=v8[:, 2 * i:2 * i + 2, :],
                            rhs=pt[:, 2 * i:2 * i + 2, llo:512],
                            perf_mode=DR, start=first, stop=last,
                            skip_group_check=True)
                        first = False
                    nc.vector.reciprocal_approx_fast(
                        isf[g][:, 512 * n:512 * (n + 1)], sgp[:])
                    bcn = bpool.tile([128, 512], f32, tag="bcn", bufs=2, name="bcn")
                    nc.gpsimd.partition_broadcast(
                        bcn[:], isf[g][:, 512 * n:512 * (n + 1)])
                    nc.vector.tensor_mul(
                        yT[g][:, 512 * n:512 * (n + 1)], yp[:], bcn[:])
                    if dbg and g == 0 and n == 0:
                        dpp = bpool.tile([128, 512], f32, tag="dpp", bufs=1, name="dpp")
                        nc.vector.tensor_copy(dpp[:], pt[:, 0:1, :])
                        nc.sync.dma_start(dp_d[:], dpp[:])

                out_sb = [bpool.tile([128, C], bf16, tag="osb", bufs=3, name="osb")
                          for _ in range(MT)]

                def cproj_cn(m, cns, late):
                    for cn in cns:
                        op = ypool.tile([128, 512], f32, tag="y", bufs=2)
                        for g in range(G):
                            nc.tensor.matmul(
                                op[:], lhsT=yT[g][:, 128 * m:128 * (m + 1)],
                                rhs=wpt[:, g:g + 1, 512 * cn:512 * (cn + 1)],
                                start=(g == 0), stop=(g == G - 1))
                        if late and cn % 2 == 1:
                            nc.scalar.activation(
                                out_sb[m][:, 512 * cn:512 * (cn + 1)], op[:],
                                AF.Copy, scale=1.0)
                        else:
                            nc.vector.tensor_copy(
                                out_sb[m][:, 512 * cn:512 * (cn + 1)], op[:])
                    if late:
                        for cn in cns:
                            nc.sync.dma_start(
                                out_d[128 * m:128 * (m + 1), 512 * cn:512 * (cn + 1)],
                                out_sb[m][:, 512 * cn:512 * (cn + 1)])
                    elif cns[-1] == NT - 1:
                        nc.sync.dma_start(
                            out_d[128 * m:128 * (m + 1), :], out_sb[m][:])

                for n in range(NT):
                    if n == 0:
                        pts = []
                        for g in range(G):
                            p00[g] = bpool.tile([128, 128], bf16, tag="p00",
                                                bufs=4, name="p00")
                            pt = bpool.tile([128, 4, 512], f8,
                                            tag="pt0", bufs=4, name="pt")
                            pts.append(pt)
                            scores_exp(g, 0, pt)
                        for g in range(G):
                            sig_pv(g, 0, pts[g])
                        continue
                    for g in range(G):
                        pt = bpool.tile([128, 4 * n + 4, 512], f8,
                                        tag=f"pt{n}", bufs=2, name="pt")
                        scores_exp(g, n, pt)
                        cproj_cn(4 * (n - 1) + g, [0, 1], late=False)
                        sig_pv(g, n, pt)
                        cproj_cn(4 * (n - 1) + g, [2, 3], late=False)
                    if n == 3:
                        for m in range(12, 16):
                            cproj_cn(m, [0, 1], late=True)
                            cproj_cn(m, [2, 3], late=True)
                if dbg:
                    nc.sync.dma_start(dis_d[:], isf[0][:])
                    dyt = bpool.tile([128, T], f32, tag="dyt", bufs=1, name="dyt")
                    nc.vector.tensor_copy(dyt[:], yT[0][:])
                    nc.sync.dma_start(dyt_d[:], dyt[:])

    nc.finalize()
    return nc


def _f8(x):
    return np.clip(x, -240.0, 240.0).astype(F8)


def host_inputs(x, Wq, bq, Wkv, bkv, Wproj):
    af = (1.0 / 1024.0) ** np.linspace(0.0, 1.0, D // 4, dtype=np.float32)
    af = np.concatenate([af, np.zeros(D // 4, dtype=np.float32)])  # (64,)
    ident = np.eye(128, dtype=np.float32).astype(BF16)
    p = np.arange(128)
    nm = np.where(p[None, :] >= p[:, None], 0.0, -30.0).astype(BF16)

    # x layouts per batch: xt [128, KC*2048] hi, xlo [128, KC*128]
    xts, xlos = [], []
    for b in range(B):
        xs = np.ascontiguousarray(x[b].T) * SX          # (C, T) scaled
        xc = xs.reshape(KC, 128, T)                      # chunk, p, t
        hi = _f8(xc)
        xts.append(np.ascontiguousarray(
            hi.transpose(1, 0, 2).reshape(128, KC * T)))
        lo = _f8((xc[:, :, :128] - hi[:, :, :128].astype(np.float32)))
        xlos.append(np.ascontiguousarray(
            lo.transpose(1, 0, 2).reshape(128, KC * 128)))

    def wsplit(Wm):
        """Wm (C, width) scaled -> ([128, KC*width] hi, [128, KC*width] lo)."""
        width = Wm.shape[1]
        wc = (Wm * SW).reshape(KC, 128, width)
        hi = _f8(wc)
        lo = _f8(wc - hi.astype(np.float32))
        mk = lambda a: np.ascontiguousarray(
            a.transpose(1, 0, 2).reshape(128, KC * width))
        return mk(hi), mk(lo)

    def bias_rows(bvec):
        t = bvec * (S / 128.0)
        hi = _f8(t)
        lo = _f8(t - hi.astype(np.float32))
        return hi, lo

    in_maps = []
    for core in range(8):
        b, j = core // 4, core % 4
        wq_parts, bq_parts = [], []
        for g in range(G):
            h = G * j + g
            th = (h - j) * af
            cth, sth = np.cos(th).astype(np.float32), np.sin(th).astype(np.float32)
            R = np.zeros((D, D), np.float32)
            i = np.arange(64)
            R[i, i] = cth
            R[i, 64 + i] = sth
            R[64 + i, i] = -sth
            R[64 + i, 64 + i] = cth
            wq_parts.append(Wq[:, h * D:(h + 1) * D] @ R.T)
            bq_parts.append(bq[h * D:(h + 1) * D] @ R.T)
        Wq_j = np.concatenate(wq_parts, axis=1)          # (C, 512)
        Wk_j = Wkv[:, j * D:(j + 1) * D]
        Wv_j = Wkv[:, N_KV * D + j * D:N_KV * D + (j + 1) * D]
        bq_j = np.concatenate(bq_parts)
        bk_j = bkv[j * D:(j + 1) * D]
        bv_j = bkv[N_KV * D + j * D:N_KV * D + (j + 1) * D]
        bhi, blo = bias_rows(np.concatenate([bq_j, bk_j, bv_j]))   # (768,)
        bw = np.ascontiguousarray(np.broadcast_to(
            np.stack([bhi, blo], axis=0)[None, :, :], (128, 2, 768)
        ).reshape(128, 2 * 768))
        wp = np.ascontiguousarray(
            Wproj[512 * j:512 * (j + 1), :].reshape(G, 128, C)
            .transpose(1, 0, 2).reshape(128, G * C)).astype(BF16)
        wqh_a, wql_a = wsplit(Wq_j)
        wkh_a, wkl_a = wsplit(Wk_j)
        wvh_a, wvl_a = wsplit(Wv_j)
        in_maps.append({
            "xt": xts[b], "xlo": xlos[b],
            "wqh": wqh_a, "wql": wql_a, "wkh": wkh_a, "wkl": wkl_a,
            "wvh": wvh_a, "wvl": wvl_a,
            "wp": wp, "bw": bw,
            "ident": ident, "nm": nm,
        })
    return in_maps


def assemble(parts, bproj):
    out = np.empty((B, T, C), np.float32)
    for b in range(B):
        out[b] = parts[4 * b].astype(np.float32)
        for c in range(1, 4):
            out[b] += parts[4 * b + c].astype(np.float32)
        out[b] += bproj[None, :]
    return out


def kernel(x, mask, Wq, bq, Wkv, bkv, Wproj, bproj):
    from concourse.bass_utils import run_bass_kernel_spmd

    x = np.asarray(x, np.float32)
    in_maps = host_inputs(
        x, np.asarray(Wq, np.float32), np.asarray(bq, np.float32),
        np.asarray(Wkv, np.float32), np.asarray(bkv, np.float32),
        np.asarray(Wproj, np.float32))
    if "nc" not in _CACHE:
        _CACHE["nc"] = build_nc()
    res = run_bass_kernel_spmd(_CACHE["nc"], in_maps, list(range(8)))
    parts = [res.results[c]["out"] for c in range(8)]
    return assemble(parts, np.asarray(bproj, np.float32))


# revision 5
# speedup vs baseline: 1.0229x; 1.0005x over previous
"""Trainium2 Bass kernel for nn_Attention_35588099015465 — fp8 DoubleRow version.

Full GQA attention layer (QKV proj + per-head RMS norm + head-indexed rotary +
causal SDPA + out proj), sharded over 8 NeuronCores as DP(batch=2) x TP(kv=4).

Speed structure (vs the bf16 baseline):
  - QKV projections run as fp8e4m3 DoubleRow matmuls (K=256 packed per
    instruction, 0.5 cyc/col): pure-fp8 main products + hi/lo residual
    corrections restricted to the first 128 tokens.  Softmax averaging
    attenuates quantization noise ~1/sqrt(row length), so only short-context
    rows need the compensation (measured: full-pure rel-err 0.047 -> 0.0088
    with the 128-row prefix).
  - Biases enter via an extra DoubleRow matmul against a ones vector
    (hi/lo fp8 split keeps them exact to ~0.4%); evacuation rides ACT.
  - P = exp(S - 2) is written directly in fp8 (e4m3, bias -2 keeps the
    range in [0, 54]); Sigma and P@V run as DoubleRow with chunk-paired P.
    The first 128 tq columns use a bf16 P block + bf16 v chunk instead.
  - Scores and the output projection stay bf16 (q/k direction noise and
    y/Wproj quantization do not attenuate; measured over budget in fp8).
  - Phase B iterates tq-chunk-major (n outer, head inner) so the output
    projection + its DMA interleave with the ACT-bound softmax work.
"""

import numpy as np
import ml_dtypes

B, T, C = 2, 2048, 2048
N_HEAD, N_KV = 16, 4
D = 128
G = N_HEAD // N_KV  # 4
EPS = 1.1920928955078125e-07
KC = C // 128  # 16 contraction chunks
MT = T // 128  # 16 row chunks
NT = T // 512  # 4 col chunks
SX, SW, SV = 16.0, 1024.0, 32.0
S = SX * SW
BIAS_Q = S * S * D * EPS  # 4096.0 exactly
BIAS_K = S * S * EPS      # 32.0 exactly

F8 = ml_dtypes.float8_e4m3
BF16 = ml_dtypes.bfloat16

_CACHE = {}


def build_nc(dbg=False):
    import concourse.mybir as mybir
    import concourse.tile as tile
    from concourse import bacc

    dt = mybir.dt
    f32, bf16, f8 = dt.float32, dt.bfloat16, dt.float8e4
    AF = mybir.ActivationFunctionType
    DR = mybir.MatmulPerfMode.DoubleRow

    nc = bacc.Bacc("TRN2", target_bir_lowering=False, debug=False, num_devices=8)

    xt_d = nc.declare_dram_parameter("xt", [128, KC * 2048], f8, isOutput=False)
    xlo_d = nc.declare_dram_parameter("xlo", [128, KC * 128], f8, isOutput=False)
    wqh_d = nc.declare_dram_parameter("wqh", [128, KC * 512], f8, isOutput=False)
    wql_d = nc.declare_dram_parameter("wql", [128, KC * 512], f8, isOutput=False)
    wkh_d = nc.declare_dram_parameter("wkh", [128, KC * 128], f8, isOutput=False)
    wkl_d = nc.declare_dram_parameter("wkl", [128, KC * 128], f8, isOutput=False)
    wvh_d = nc.declare_dram_parameter("wvh", [128, KC * 128], f8, isOutput=False)
    wvl_d = nc.declare_dram_parameter("wvl", [128, KC * 128], f8, isOutput=False)
    wp_d = nc.declare_dram_parameter("wp", [128, G * 2048], bf16, isOutput=False)
    bw_d = nc.declare_dram_parameter("bw", [128, 2 * 768], f8, isOutput=False)
    ident_d = nc.declare_dram_parameter("ident", [128, 128], bf16, isOutput=False)
    nm_d = nc.declare_dram_parameter("nm", [128, 128], bf16, isOutput=False)
    out_d = nc.declare_dram_parameter("out", [T, C], bf16, isOutput=True)
    if dbg:
        dqh_d = nc.declare_dram_parameter("dqh", [128, T], f32, isOutput=True)
        dkh_d = nc.declare_dram_parameter("dkh", [128, T], f32, isOutput=True)
        dv_d = nc.declare_dram_parameter("dv", [128, D], f32, isOutput=True)
        dp_d = nc.declare_dram_parameter("dp", [128, 512], f32, isOutput=True)
        dis_d = nc.declare_dram_parameter("dis", [1, T], f32, isOutput=True)
        dyt_d = nc.declare_dram_parameter("dyt", [128, T], f32, isOutput=True)

    with tile.TileContext(nc) as tc:
        with (
            tc.tile_pool(name="consts", bufs=1) as cpool,
            tc.tile_pool(name="persist", bufs=1) as ppool,
        ):
            ident = cpool.tile([128, 128], bf16, tag="ident")
            nc.sync.dma_start(ident[:], ident_d[:])
            nm = cpool.tile([128, 128], bf16, tag="nm")
            nc.sync.dma_start(nm[:], nm_d[:])
            bw = cpool.tile([128, 2, 768], f8, tag="bw")
            nc.sync.dma_start(bw[:], bw_d[:])
            ones_mov = cpool.tile([128, 2, 512], f8, tag="ones_mov")
            nc.vector.memset(ones_mov[:], 1.0)
            ones1 = cpool.tile([128, 1], bf16, tag="ones1")
            nc.vector.memset(ones1[:], 1.0)
            onesv = cpool.tile([128, 1], bf16, tag="onesv")
            nc.vector.memset(onesv[:], SV)
            ones8 = cpool.tile([128, 2, 16], f8, tag="ones8")
            nc.vector.memset(ones8[:], SV)
            nbias = cpool.tile([128, 1], f32, tag="nbias")
            nc.vector.memset(nbias[:], -2.0)
            biasq = cpool.tile([1, 1], f32, tag="biasq")
            nc.vector.memset(biasq[:], BIAS_Q)
            biask = cpool.tile([1, 1], f32, tag="biask")
            nc.vector.memset(biask[:], BIAS_K)

            # persistent across phases
            qh = [ppool.tile([128, T], bf16, tag="qh", bufs=G, name="qh") for _ in range(G)]
            kh = ppool.tile([128, T], bf16, tag="kh", name="kh")
            v8 = ppool.tile([128, KC, D], f8, tag="v8", name="v8")
            v0b = ppool.tile([128, D], bf16, tag="v0b", name="v0b")
            yT = [ppool.tile([128, T], bf16, tag="yT", bufs=G, name="yT") for _ in range(G)]
            wpt = ppool.tile([128, G, 2048], bf16, tag="wpt", name="wpt")

            # ---------------- Phase A: projections + norms ----------------
            with (
                tc.tile_pool(name="phA", bufs=1) as apool,
                tc.tile_pool(name="phA_ps", space="PSUM", bufs=4) as aps,
                tc.tile_pool(name="phA_ss", space="PSUM", bufs=2) as sps_pool,
                tc.tile_pool(name="phA_tp", space="PSUM", bufs=2) as tp_pool,
            ):
                xt = apool.tile([128, KC, 2048], f8, tag="xt", name="xt")
                xlo = apool.tile([128, KC, 128], f8, tag="xlo", name="xlo")
                wqh = apool.tile([128, KC, 512], f8, tag="wqh", name="wqh")
                wql = apool.tile([128, KC, 512], f8, tag="wql", name="wql")
                wkh = apool.tile([128, KC, 128], f8, tag="wkh", name="wkh")
                wkl = apool.tile([128, KC, 128], f8, tag="wkl", name="wkl")
                wvh = apool.tile([128, KC, 128], f8, tag="wvh", name="wvh")
                wvl = apool.tile([128, KC, 128], f8, tag="wvl", name="wvl")
                for k2 in range(KC // 2):
                    nc.sync.dma_start(xt[:, 2 * k2:2 * k2 + 2, :],
                                      xt_d[:, 4096 * k2:4096 * (k2 + 1)])
                    if k2 % 2 == 0:
                        nc.sync.dma_start(wqh[:, 2 * k2:2 * k2 + 4, :],
                                          wqh_d[:, 1024 * k2:1024 * k2 + 2048])
                nc.sync.dma_start(xlo[:], xlo_d[:])
                nc.sync.dma_start(wkh[:], wkh_d[:])
                nc.sync.dma_start(wvh[:], wvh_d[:])
                nc.sync.dma_start(wql[:], wql_d[:])
                nc.sync.dma_start(wkl[:], wkl_d[:])
                nc.sync.dma_start(wvl[:], wvl_d[:])
                for g in range(G):
                    nc.sync.dma_start(wpt[:, g:g + 1, :],
                                      wp_d[:, 2048 * g:2048 * (g + 1)])

                # PE warm-up during the input-DMA ramp
                for w in range(36):
                    wps = aps.tile([128, 512], f32, tag="proj", bufs=4, name="wps")
                    nc.tensor.matmul(wps[:, :128], lhsT=ident[:], rhs=ident[:],
                                     start=True, stop=True)

                def project(unit, dst, evac_scale):
                    """unit 0..3 = q head g, 4 = k, 5 = v; dst [128, T] bf16."""
                    if unit < G:
                        wth, wtl, hi0, bcol = wqh, wql, 128 * unit, 128 * unit
                    elif unit == 4:
                        wth, wtl, hi0, bcol = wkh, wkl, 0, 512
                    else:
                        wth, wtl, hi0, bcol = wvh, wvl, 0, 640
                    for n in range(NT):
                        ps = aps.tile([128, 512], f32, tag="proj", bufs=4)
                        nc.tensor.matmul(
                            ps[:], lhsT=bw[:, :, bcol:bcol + 128],
                            rhs=ones_mov[:], perf_mode=DR,
                            start=True, stop=False, skip_group_check=True)
                        for i in range(KC // 2):
                            nc.tensor.matmul(
                                ps[:], lhsT=wth[:, 2 * i:2 * i + 2, hi0:hi0 + 128],
                                rhs=xt[:, 2 * i:2 * i + 2, 512 * n:512 * (n + 1)],
                                perf_mode=DR, start=False,
                                stop=(n > 0 and i == KC // 2 - 1),
                                skip_group_check=True)
                        if n == 0:
                            for i in range(KC // 2):
                                nc.tensor.matmul(
                                    ps[:, 0:128],
                                    lhsT=wtl[:, 2 * i:2 * i + 2, hi0:hi0 + 128],
                                    rhs=xt[:, 2 * i:2 * i + 2, 0:128],
                                    perf_mode=DR, start=False, stop=False,
                                    skip_group_check=True)
                            for i in range(KC // 2):
                                nc.tensor.matmul(
                                    ps[:, 0:128],
                                    lhsT=wth[:, 2 * i:2 * i + 2, hi0:hi0 + 128],
                                    rhs=xlo[:, 2 * i:2 * i + 2, :],
                                    perf_mode=DR, start=False,
                                    stop=(i == KC // 2 - 1),
                                    skip_group_check=True)
                        nc.scalar.activation(
                            dst[:, 512 * n:512 * (n + 1)], ps[:], AF.Copy,
                            scale=evac_scale)

                def norm_apply(src, dst, bias_ap):
                    """dst = src * broadcast(1/sqrt(scale-corrected sumsq)),
                    chunked per 512 cols so the chain pipelines across engines."""
                    sq = apool.tile([128, T], bf16, tag="sq", bufs=2, name="sq")
                    nc.vector.tensor_mul(sq[:], src[:], src[:])
                    srow = apool.tile([1, T], f32, tag="srow", bufs=2, name="srow")
                    crow = apool.tile([1, T], f32, tag="crow", bufs=2, name="crow")
                    for n in range(NT):
                        ssp = sps_pool.tile([1, 512], f32, tag="ss", bufs=2)
                        nc.tensor.matmul(
                            ssp[:], lhsT=ones1[:], rhs=sq[:, 512 * n:512 * (n + 1)],
                            start=True, stop=True)
                        nc.scalar.activation(
                            srow[:, 512 * n:512 * (n + 1)], ssp[:], AF.Sqrt,
                            bias=bias_ap, scale=(1.0 if bias_ap is biasq else 1.0 / D))
                        nc.vector.reciprocal_approx_fast(
                            crow[:, 512 * n:512 * (n + 1)],
                            srow[:, 512 * n:512 * (n + 1)])
                        bcc = apool.tile([128, 512], f32, tag="bcc", bufs=3, name="bcc")
                        nc.gpsimd.partition_broadcast(
                            bcc[:], crow[:, 512 * n:512 * (n + 1)])
                        nc.vector.tensor_mul(
                            dst[:, 512 * n:512 * (n + 1)],
                            src[:, 512 * n:512 * (n + 1)], bcc[:])

                # software-pipelined: project unit u+1 before norming unit u
                unit_order = [0, 4, 1, 2, 3, 5]   # q0, K, q1, q2, q3, V
                dsts, pending = {}, None
                for unit in unit_order:
                    if unit == 5:
                        vT = apool.tile([128, T], bf16, tag="vT", name="vT")
                        dsts[5] = vT
                        project(5, vT, SV / S)
                    else:
                        srcu = apool.tile([128, T], bf16, tag="qsb", bufs=3, name="qsb")
                        dsts[unit] = srcu
                        project(unit, srcu, 1.0)
                    if unit == 5:
                        for m in range(MT):
                            tp = tp_pool.tile([128, 128], bf16, tag="vtp", bufs=2)
                            nc.tensor.transpose(
                                tp[:], vT[:, 128 * m:128 * (m + 1)], ident[:])
                            if m % 2 == 0:
                                nc.vector.tensor_copy(v8[:, m:m + 1, :], tp[:])
                            else:
                                nc.scalar.activation(
                                    v8[:, m:m + 1, :], tp[:], AF.Copy, scale=1.0)
                            if m == 0:
                                nc.vector.tensor_copy(v0b[:], tp[:])
                    if pending is not None:
                        if pending == 4:
                            norm_apply(dsts[4], kh, biask)
                        else:
                            norm_apply(dsts[pending], qh[pending], biasq)
                    pending = unit
                # preload the exp table during phase A's ACT idle
                dume = apool.tile([1, 1], f32, tag="dume", bufs=1, name="dume")
                nc.scalar.activation(dume[:], biasq[:], AF.Exp)
                if dbg:
                    dcp = apool.tile([128, T], f32, tag="dcp", bufs=1, name="dcp")
                    nc.vector.tensor_copy(dcp[:], qh[0][:])
                    nc.sync.dma_start(dqh_d[:], dcp[:])
                    dcp2 = apool.tile([128, T], f32, tag="dcp2", bufs=1, name="dcp2")
                    nc.vector.tensor_copy(dcp2[:], kh[:])
                    nc.sync.dma_start(dkh_d[:], dcp2[:])
                    dcp3 = apool.tile([128, D], f32, tag="dcp3", bufs=1, name="dcp3")
                    nc.vector.tensor_copy(dcp3[:], v8[:, 0:1, :])
                    nc.sync.dma_start(dv_d[:], dcp3[:])

            # ---------------- Phase B: attention + interleaved out-proj ----------------
            with (
                tc.tile_pool(name="phB", bufs=1) as bpool,
                tc.tile_pool(name="phB_s", space="PSUM", bufs=2) as spool,
                tc.tile_pool(name="phB_y", space="PSUM", bufs=3) as ypool,
                tc.tile_pool(name="phB_sg", space="PSUM", bufs=1) as sgpool,
            ):
                isf = [bpool.tile([1, T], f32, tag="isf", bufs=G, name="isf") for _ in range(G)]

                # P00s: bf16 prefix P block per head (cols 0:512, only 0:128 nonzero)
                p00 = {}

                def scores_exp(g, n, pt):
                    """scores+exp for head g, tq window n, into pt [128, 4n+4, 512]."""
                    kmax = 4 * n + 3
                    for i in range(2 * n + 2):
                        sp = spool.tile([128, 2, 512], f32, tag="sp", bufs=2, name="sp")
                        for j in (0, 1):
                            kk = 2 * i + j
                            lo = max(128 * kk, 512 * n)
                            diag = 4 * n <= kk <= kmax
                            nc.tensor.matmul(
                                sp[:, j:j + 1, lo - 512 * n:512],
                                lhsT=kh[:, 128 * kk:128 * (kk + 1)],
                                rhs=qh[g][:, lo:512 * (n + 1)],
                                start=True, stop=not diag,
                                skip_group_check=True)
                            if diag:
                                dcol = 128 * kk - 512 * n
                                nc.tensor.matmul(
                                    sp[:, j:j + 1, dcol:dcol + 128],
                                    lhsT=ident[:], rhs=nm[:],
                                    start=False, stop=True,
                                    skip_group_check=True)
                        if i < 2 * n:
                            nc.scalar.activation(
                                pt[:, 2 * i:2 * i + 2, :], sp[:], AF.Exp,
                                bias=nbias[:])
                        else:
                            for j in (0, 1):
                                kk = 2 * i + j
                                lo = 128 * kk - 512 * n
                                nc.scalar.activation(
                                    pt[:, kk:kk + 1, lo:512], sp[:, j:j + 1, lo:512],
                                    AF.Exp, bias=nbias[:])
                                if kk == 0 and n == 0:
                                    ph = p00[g]
                                    nc.scalar.activation(
                                        ph[:, 0:128], sp[:, 0:1, 0:128], AF.Exp,
                                        bias=nbias[:])
                    # zero subdiagonal gaps of the two diagonal pairs
                    nc.gpsimd.memset(pt[:, 4 * n + 1:4 * n + 2, 0:128], 0.0)
                    nc.gpsimd.memset(pt[:, 4 * n + 3:4 * n + 4, 256:384], 0.0)

                def sig_pv(g, n, pt):
                    sgp = sgpool.tile([1, 512], f32, tag="sg", bufs=1)
                    yp = ypool.tile([128, 512], f32, tag="y", bufs=3)
                    first = True
                    if n == 0:
                        ph = p00[g]
                        nc.tensor.matmul(sgp[:, 0:128], lhsT=onesv[:], rhs=ph[:],
                                         start=True, stop=False,
                                         skip_group_check=True)
                        nc.tensor.matmul(yp[:, 0:128], lhsT=v0b[:], rhs=ph[:],
                                         start=True, stop=False,
                                         skip_group_check=True)
                        first = False
                    for i in range(2 * n + 2):
                        lo = max(256 * i, 512 * n, 128)
                        llo = lo - 512 * n
                        last = (i == 2 * n + 1)
                        nc.tensor.matmul(
                            sgp[:, llo:512], lhsT=ones8[:, :, 0:1],
                            rhs=pt[:, 2 * i:2 * i + 2, llo:512],
                            perf_mode=DR, start=first, stop=last,
                            skip_group_check=True)
                        nc.tensor.matmul(
                            yp[:, llo:512], lhsT=v8[:, 2 * i:2 * i + 2, :],
                            rhs=pt[:, 2 * i:2 * i + 2, llo:512],
                            perf_mode=DR, start=first, stop=last,
                            skip_group_check=True)
                        first = False
                    nc.vector.reciprocal_approx_fast(
                        isf[g][:, 512 * n:512 * (n + 1)], sgp[:])
                    bcn = bpool.tile([128, 512], f32, tag="bcn", bufs=2, name="bcn")
                    nc.gpsimd.partition_broadcast(
                        bcn[:], isf[g][:, 512 * n:512 * (n + 1)])
                    nc.vector.tensor_mul(
                        yT[g][:, 512 * n:512 * (n + 1)], yp[:], bcn[:])
                    if dbg and g == 0 and n == 0:
                        dpp = bpool.tile([128, 512], f32, tag="dpp", bufs=1, name="dpp")
                        nc.vector.tensor_copy(dpp[:], pt[:, 0:1, :])
                        nc.sync.dma_start(dp_d[:], dpp[:])

                out_sb = [bpool.tile([128, C], bf16, tag="osb", bufs=3, name="osb")
                          for _ in range(MT)]

                def cproj_cn(m, cns, late):
                    for cn in cns:
                        op = ypool.tile([128, 512], f32, tag="y", bufs=3)
                        for g in range(G):
                            nc.tensor.matmul(
                                op[:], lhsT=yT[g][:, 128 * m:128 * (m + 1)],
                                rhs=wpt[:, g:g + 1, 512 * cn:512 * (cn + 1)],
                                start=(g == 0), stop=(g == G - 1))
                        if late and cn % 2 == 1:
                            nc.scalar.activation(
                                out_sb[m][:, 512 * cn:512 * (cn + 1)], op[:],
                                AF.Copy, scale=1.0)
                        else:
                            nc.vector.tensor_copy(
                                out_sb[m][:, 512 * cn:512 * (cn + 1)], op[:])
                    if late:
                        for cn in cns:
                            nc.sync.dma_start(
                                out_d[128 * m:128 * (m + 1), 512 * cn:512 * (cn + 1)],
                                out_sb[m][:, 512 * cn:512 * (cn + 1)])
                    elif cns[-1] == NT - 1:
                        nc.sync.dma_start(
                            out_d[128 * m:128 * (m + 1), :], out_sb[m][:])

                for n in range(NT):
                    if n == 0:
                        pts = []
                        for g in range(G):
                            p00[g] = bpool.tile([128, 128], bf16, tag="p00",
                                                bufs=4, name="p00")
                            pt = bpool.tile([128, 4, 512], f8,
                                            tag="pt0", bufs=4, name="pt")
                            pts.append(pt)
                            scores_exp(g, 0, pt)
                        for g in range(G):
                            sig_pv(g, 0, pts[g])
                        continue
                    for g in range(G):
                        pt = bpool.tile([128, 4 * n + 4, 512], f8,
                                        tag=f"pt{n}", bufs=2, name="pt")
                        scores_exp(g, n, pt)
                        cproj_cn(4 * (n - 1) + g, [0, 1], late=False)
                        sig_pv(g, n, pt)
                        cproj_cn(4 * (n - 1) + g, [2, 3], late=False)
                    if n == 3:
                        for m in range(12, 16):
                            cproj_cn(m, [0, 1], late=True)
                            cproj_cn(m, [2, 3], late=True)
                if dbg:
                    nc.sync.dma_start(dis_d[:], isf[0][:])
                    dyt = bpool.tile([128, T], f32, tag="dyt", bufs=1, name="dyt")
                    nc.vector.tensor_copy(dyt[:], yT[0][:])
                    nc.sync.dma_start(dyt_d[:], dyt[:])

    nc.finalize()
    return nc


def _f8(x):
    return np.clip(x, -240.0, 240.0).astype(F8)


def host_inputs(x, Wq, bq, Wkv, bkv, Wproj):
    af = (1.0 / 1024.0) ** np.linspace(0.0, 1.0, D // 4, dtype=np.float32)
    af = np.concatenate([af, np.zeros(D // 4, dtype=np.float32)])  # (64,)
    ident = np.eye(128, dtype=np.float32).astype(BF16)
    p = np.arange(128)
    nm = np.where(p[None, :] >= p[:, None], 0.0, -30.0).astype(BF16)

    # x layouts per batch: xt [128, KC*2048] hi, xlo [128, KC*128]
    xts, xlos = [], []
    for b in range(B):
        xs = np.ascontiguousarray(x[b].T) * SX          # (C, T) scaled
        xc = xs.reshape(KC, 128, T)                      # chunk, p, t
        hi = _f8(xc)
        xts.append(np.ascontiguousarray(
            hi.transpose(1, 0, 2).reshape(128, KC * T)))
        lo = _f8((xc[:, :, :128] - hi[:, :, :128].astype(np.float32)))
        xlos.append(np.ascontiguousarray(
            lo.transpose(1, 0, 2).reshape(128, KC * 128)))

    def wsplit(Wm):
        """Wm (C, width) scaled -> ([128, KC*width] hi, [128, KC*width] lo)."""
        width = Wm.shape[1]
        wc = (Wm * SW).reshape(KC, 128, width)
        hi = _f8(wc)
        lo = _f8(wc - hi.astype(np.float32))
        mk = lambda a: np.ascontiguousarray(
            a.transpose(1, 0, 2).reshape(128, KC * width))
        return mk(hi), mk(lo)

    def bias_rows(bvec):
        t = bvec * (S / 128.0)
        hi = _f8(t)
        lo = _f8(t - hi.astype(np.float32))
        return hi, lo

    in_maps = []
    for core in range(8):
        b, j = core // 4, core % 4
        wq_parts, bq_parts = [], []
        for g in range(G):
            h = G * j + g
            th = (h - j) * af
            cth, sth = np.cos(th).astype(np.float32), np.sin(th).astype(np.float32)
            R = np.zeros((D, D), np.float32)
            i = np.arange(64)
            R[i, i] = cth
            R[i, 64 + i] = sth
            R[64 + i, i] = -sth
            R[64 + i, 64 + i] = cth
            wq_parts.append(Wq[:, h * D:(h + 1) * D] @ R.T)
            bq_parts.append(bq[h * D:(h + 1) * D] @ R.T)
        Wq_j = np.concatenate(wq_parts, axis=1)          # (C, 512)
        Wk_j = Wkv[:, j * D:(j + 1) * D]
        Wv_j = Wkv[:, N_KV * D + j * D:N_KV * D + (j + 1) * D]
        bq_j = np.concatenate(bq_parts)
        bk_j = bkv[j * D:(j + 1) * D]
        bv_j = bkv[N_KV * D + j * D:N_KV * D + (j + 1) * D]
        bhi, blo = bias_rows(np.concatenate([bq_j, bk_j, bv_j]))   # (768,)
        bw = np.ascontiguousarray(np.broadcast_to(
            np.stack([bhi, blo], axis=0)[None, :, :], (128, 2, 768)
        ).reshape(128, 2 * 768))
        wp = np.ascontiguousarray(
            Wproj[512 * j:512 * (j + 1), :].reshape(G, 128, C)
            .transpose(1, 0, 2).reshape(128, G * C)).astype(BF16)
        wqh_a, wql_a = wsplit(Wq_j)
        wkh_a, wkl_a = wsplit(Wk_j)
        wvh_a, wvl_a = wsplit(Wv_j)
        in_maps.append({
            "xt": xts[b], "xlo": xlos[b],
            "wqh": wqh_a, "wql": wql_a, "wkh": wkh_a, "wkl": wkl_a,
            "wvh": wvh_a, "wvl": wvl_a,
            "wp": wp, "bw": bw,
            "ident": ident, "nm": nm,
        })
    return in_maps


def assemble(parts, bproj):
    out = np.empty((B, T, C), np.float32)
    for b in range(B):
        out[b] = parts[4 * b].astype(np.float32)
        for c in range(1, 4):
            out[b] += parts[4 * b + c].astype(np.float32)
        out[b] += bproj[None, :]
    return out


def kernel(x, mask, Wq, bq, Wkv, bkv, Wproj, bproj):
    from concourse.bass_utils import run_bass_kernel_spmd

    x = np.asarray(x, np.float32)
    in_maps = host_inputs(
        x, np.asarray(Wq, np.float32), np.asarray(bq, np.float32),
        np.asarray(Wkv, np.float32), np.asarray(bkv, np.float32),
        np.asarray(Wproj, np.float32))
    if "nc" not in _CACHE:
        _CACHE["nc"] = build_nc()
    res = run_bass_kernel_spmd(_CACHE["nc"], in_maps, list(range(8)))
    parts = [res.results[c]["out"] for c in range(8)]
    return assemble(parts, np.asarray(bproj, np.float32))
